# revision 11
# baseline (speedup 1.0000x reference)
"""Trainium2 Bass kernel for nn_EmotionalEmbeddingSpace.

Sharding: data-parallel over batch B=16 across 8 cores (2 sequences/core).
Layout on device: transposed — features on partitions, tokens on the free dim.

Per core (BL=2 sequences, NTOK=BL*S tokens):
  pt   = Wm^T @ x^T + bm                       (bulk matmul, bf16 in, f32 psum)
  mem_j = tanh(pt_j + Um^T @ mem_{j-1})        (serial over S steps; Um stationary
                                                bf16 tiles, mem columns moving)
  latx = encode(x), latm = encode(mem)         (token-parallel MLP chains, chunked)
  recon/trans/ctx per-token losses -> tok_loss[NTOK] -> host combines + l2 term.

Matmul inputs are bf16 (PSUM accumulation in f32); LN statistics and all loss
math in f32.
"""

import sys

sys.path.insert(0, "/opt/trn_rl_repo")

import numpy as np
import ml_dtypes

import concourse.bass as bass
import concourse.bacc as bacc
import concourse.mybir as mybir
import concourse.tile as tile
from concourse.bass_utils import run_bass_kernel_spmd

F32 = mybir.dt.float32
BF16 = mybir.dt.bfloat16
AF = mybir.ActivationFunctionType
ALU = mybir.AluOpType

B, S_FULL, D, H, L = 16, 1024, 768, 512, 128
NCORES = 8
LN_EPS = 1e-5
NORM_EPS = 1e-8

# loss row indices in the packed [8, NTOK] rows tile
R_RECON, R_TRANS, R_NX, R_NM, R_DOT, R_CTX, R_TOKS, R_TMP = range(8)


# ---------------------------------------------------------------- host prep

def _pack_cols(*vecs):
    """Pack per-feature vectors into a [128, ncols] f32 matrix: each vector of
    length n*128 becomes n columns (col t = features t*128..t*128+127)."""
    cols = []
    for v in vecs:
        v = np.asarray(v, np.float32).reshape(-1, 128)
        cols.append(v.T)
    return np.ascontiguousarray(np.concatenate(cols, axis=1))


def _ln_np(x, g, b, eps=LN_EPS):
    m = x.mean(-1, keepdims=True)
    v = ((x - m) ** 2).mean(-1, keepdims=True)
    return (x - m) / np.sqrt(v + eps) * g + b


def _encode_np(t, w):
    h = np.maximum(_ln_np(t @ w["W1"] + w["b1"], w["g1"], w["be1"]), 0)
    a = h @ w["Wvo"] + w["bvo"]
    g = np.maximum(_ln_np(a @ w["W2"] + w["b2"], w["g2"], w["be2"]), 0)
    zl = _ln_np(g @ w["W3"] + w["b3"], w["g3"], w["be3"])
    e = np.maximum(_ln_np(zl @ w["W4"] + w["b4"], w["g4"], w["be4"]), 0)
    return _ln_np(e @ w["W5"] + w["b5"], w["g5"], w["be5"])


# ---------------------------------------------------------------- builder

class _KB:
    """Kernel builder (per-core SPMD program)."""

    def __init__(self, S=S_FULL, BL=B // NCORES):
        self.S, self.BL = S, BL
        self.NTOK = S * BL
        self.CH = min(512, self.NTOK)          # token chunk for MLP phases
        self.NCH = self.NTOK // self.CH
        self.nc = bacc.Bacc("TRN2", target_bir_lowering=False, debug=False,
                            num_devices=NCORES)
        self.vec_map = {}
        self._vec_cols = 0

    def _reg_vec(self, name, ntiles):
        self.vec_map[name] = (self._vec_cols, ntiles)
        self._vec_cols += ntiles

    def declare(self):
        nc = self.nc
        NT = self.NTOK
        self.d_xt = nc.dram_tensor("xt", [D, NT], BF16, kind="ExternalInput")
        wshapes = dict(W1=(D, H), Wvo=(H, H), W2=(H, H), W3=(H, L), W4=(L, H),
                       W5=(H, L), Wd1=(L, H), Wd2=(H, H), Wd3=(H, D),
                       Wm=(D, D), Um=(D, D))
        self.d_w = {k: nc.dram_tensor(k.lower() + "16", list(v), BF16,
                                      kind="ExternalInput")
                    for k, v in wshapes.items()}
        for nm, n in [("b1", 4), ("g1", 4), ("be1", 4), ("bvo", 4),
                      ("b2", 4), ("g2", 4), ("be2", 4),
                      ("b3", 1), ("g3", 1), ("be3", 1),
                      ("b4", 4), ("g4", 4), ("be4", 4),
                      ("b5", 1), ("g5", 1), ("be5", 1),
                      ("bd1", 4), ("gd1", 4), ("bed1", 4),
                      ("bd2", 4), ("gd2", 4), ("bed2", 4),
                      ("bd3", 6), ("bm", 6), ("z0", 1), ("lneps", 1)]:
            self._reg_vec(nm, n)
        self.d_vecs = nc.dram_tensor("vecs", [128, self._vec_cols], F32,
                                     kind="ExternalInput")
        self.d_id = nc.dram_tensor("id16", [128, 128], BF16,
                                   kind="ExternalInput")
        self.d_out = nc.dram_tensor("tok_loss", [1, NT], F32,
                                    kind="ExternalOutput")
        self.d_rec = nc.dram_tensor("rec_stage", [1, NT], F32)
        self.d_trn = nc.dram_tensor("trn_stage", [1, NT], F32)

    def vcol(self, name, t=0):
        s, n = self.vec_map[name]
        assert t < n
        return self.vecs_sb[:, s + t:s + t + 1]

    # ---- device helpers -------------------------------------------------
    def load_weight_tiles(self, pool, dram, K, M):
        nc = self.nc
        tiles = []
        for k in range(K // 128):
            t = pool.tile([128, M], BF16, tag=f"w_{dram.name}_{k}",
                          name=f"w_{dram.name}_{k}")
            nc.sync.dma_start(t[:], dram[k * 128:(k + 1) * 128, :])
            tiles.append(t)
        return tiles

    def layer_ch(self, in_aps, w_tiles, M_out, *, bias, ln=None, relu=False,
                 out_aps=None, out_tag=None, out_dtype=BF16):
        """One chunk of: out = [relu|id]( LN?( in @ W + b ) ), transposed.

        in_aps: list of [128, CH] APs (bf16). Returns list of out APs."""
        nc, CH = self.nc, self.CH
        n_k, n_m = len(in_aps), M_out // 128
        if out_aps is None:
            out_aps = [self.tmp_pool.tile([128, CH], out_dtype,
                                          tag=f"{out_tag}{m}",
                                          name=f"{out_tag}{m}")[:]
                       for m in range(n_m)]
        ys = []
        for m in range(n_m):
            ps = self.ps_pool.tile([128, CH], F32, tag="ps", name="ps")
            for k in range(n_k):
                nc.tensor.matmul(ps[:], w_tiles[k][:, m * 128:(m + 1) * 128],
                                 in_aps[k], start=(k == 0), stop=(k == n_k - 1))
            if ln is None:
                nc.scalar.activation(out_aps[m], ps[:],
                                     AF.Relu if relu else AF.Identity,
                                     bias=self.vcol(bias, m))
            else:
                y = self.tmp_pool.tile([128, CH], F32, tag=f"y{m}",
                                       name=f"y{m}")
                nc.scalar.activation(y[:], ps[:], AF.Identity,
                                     bias=self.vcol(bias, m))
                ys.append(y)
        if ln is not None:
            g_nm, be_nm = ln
            ps1 = self.ps_pool.tile([1, CH], F32, tag="st1", name="st1",
                                    bufs=2)
            ps2 = self.ps_pool.tile([1, CH], F32, tag="st2", name="st2",
                                    bufs=2)
            for m in range(n_m):
                sq = self.tmp_pool.tile([128, CH], F32, tag="sq", name="sq")
                nc.vector.tensor_mul(sq[:], ys[m][:], ys[m][:])
                nc.tensor.matmul(ps1[:], self.ones1[:, 0:1], ys[m][:],
                                 start=(m == 0), stop=(m == n_m - 1))
                nc.tensor.matmul(ps2[:], self.ones1[:, 0:1], sq[:],
                                 start=(m == 0), stop=(m == n_m - 1))
            inv_f = 1.0 / M_out
            mean = self.row_pool.tile([1, CH], F32, name="row", tag="mean")
            ra = self.row_pool.tile([1, CH], F32, name="row", tag="ra")
            rb_ = self.row_pool.tile([1, CH], F32, name="row", tag="rbr")
            nc.vector.tensor_scalar_mul(mean[:], ps1[:], inv_f)
            # ra = E[x^2] - mean^2 = var
            nc.vector.tensor_scalar_mul(ra[:], ps2[:], inv_f)
            nc.vector.scalar_tensor_tensor(rb_[:], mean[:], -1.0, mean[:],
                                           ALU.mult, ALU.mult)  # -mean^2
            nc.vector.tensor_add(ra[:], ra[:], rb_[:])
            nc.scalar.activation(ra[:], ra[:], AF.Sqrt,
                                 bias=self.vcol("lneps")[0:1])   # sd
            rstd = rb_
            nc.vector.reciprocal(rstd[:], ra[:])
            mr = ra
            nc.vector.tensor_mul(mr[:], mean[:], rstd[:])
            rb = self.tmp_pool.tile([128, CH], F32, tag="rb", name="rb")
            mrb = self.tmp_pool.tile([128, CH], F32, tag="mrb", name="mrb")
            nc.gpsimd.partition_broadcast(rb[:], rstd[:])
            nc.gpsimd.partition_broadcast(mrb[:], mr[:])
            for m in range(n_m):
                nc.vector.tensor_mul(ys[m][:], ys[m][:], rb[:])
                nc.vector.tensor_sub(ys[m][:], ys[m][:], mrb[:])
                nc.scalar.activation(out_aps[m], ys[m][:],
                                     AF.Relu if relu else AF.Identity,
                                     bias=self.vcol(be_nm, m),
                                     scale=self.vcol(g_nm, m))
        return out_aps

    def encode_ch(self, in_aps, lat_out_ap):
        """Full encode chain for one chunk; writes lat (f32) to lat_out_ap."""
        w = self.w_sb
        h = self.layer_ch(in_aps, w["W1"], H, bias="b1", ln=("g1", "be1"),
                          relu=True, out_tag="h")
        a = self.layer_ch(h, w["Wvo"], H, bias="bvo", out_tag="a")
        g = self.layer_ch(a, w["W2"], H, bias="b2", ln=("g2", "be2"),
                          relu=True, out_tag="h")
        zl = self.layer_ch(g, w["W3"], L, bias="b3", ln=("g3", "be3"),
                           out_tag="a")
        e = self.layer_ch(zl, w["W4"], H, bias="b4", ln=("g4", "be4"),
                          relu=True, out_tag="h")
        self.layer_ch(e, w["W5"], L, bias="b5", ln=("g5", "be5"),
                      out_aps=[lat_out_ap])

    def sumsq_chunk(self, a_aps, out_row_ap, scale, clip10=True, b_aps=None):
        """out_row[1,CH] = (clip?)(scale * sum_over_features(a*b)) per token."""
        nc, CH = self.nc, self.CH
        n = len(a_aps)
        ps = self.ps_pool.tile([1, CH], F32, tag="st1", name="st1", bufs=2)
        for m in range(n):
            sq = self.tmp_pool.tile([128, CH], F32, tag="sq", name="sq")
            o = b_aps[m] if b_aps is not None else a_aps[m]
            nc.vector.tensor_mul(sq[:], a_aps[m], o)
            nc.tensor.matmul(ps[:], self.ones1[:, 0:1], sq[:],
                             start=(m == 0), stop=(m == n - 1))
        if clip10:
            nc.vector.tensor_scalar(out_row_ap, ps[:], scale, 10.0,
                                    ALU.mult, ALU.min)
        else:
            nc.vector.tensor_scalar_mul(out_row_ap, ps[:], scale)

    # ---- main build -----------------------------------------------------
    def build(self):
        nc = self.nc
        NT, CH, S, BL = self.NTOK, self.CH, self.S, self.BL
        self.declare()
        import os as _os
        krep2 = int(_os.environ.get("KREP2", "1"))
        with tile.TileContext(nc) as tc:
            with (
                tc.tile_pool(name="const", bufs=1) as const_pool,
                tc.tile_pool(name="wenc", bufs=1) as wenc_pool,
                tc.tile_pool(name="big", bufs=1) as big_pool,
                tc.tile_pool(name="tmp", bufs=2) as tmp_pool,
                tc.tile_pool(name="rows", bufs=1) as row_pool,
            ):
                self.tmp_pool, self.row_pool = tmp_pool, row_pool

                # constants
                self.ones1 = const_pool.tile([128, 1], F32)
                nc.vector.memset(self.ones1[:], 1.0)
                self.vecs_sb = const_pool.tile([128, self._vec_cols], F32)
                nc.sync.dma_start(self.vecs_sb[:], self.d_vecs[:, :])

                # encode weights persist; Wm/Um/decode weights phase-scoped
                self.w_sb = {}
                for k, (K, M) in dict(W1=(D, H), Wvo=(H, H), W2=(H, H),
                                      W3=(H, L), W4=(L, H), W5=(H, L)).items():
                    self.w_sb[k] = self.load_weight_tiles(wenc_pool,
                                                          self.d_w[k], K, M)

                # big persistent tensors
                latx = big_pool.tile([128, NT], F32, tag="latx", name="latx")
                latm = big_pool.tile([128, NT], F32, tag="latm", name="latm")
                pt_cm = tc.tile_pool(name="ptp", bufs=1)
                pt_pool = pt_cm.__enter__()
                xt_cm = tc.tile_pool(name="xtp", bufs=1)
                xt_pool = xt_cm.__enter__()
                xt = [xt_pool.tile([128, NT], BF16, tag=f"xt{k}",
                                   name=f"xt{k}") for k in range(6)]
                for k in range(6):
                    nc.sync.dma_start(xt[k][:],
                                      self.d_xt[k * 128:(k + 1) * 128, :])
                ptw = pt_pool.tile([128, 6 * NT], BF16, tag="ptw",
                                   name="ptw")
                pt = [ptw[:, m * NT:(m + 1) * NT] for m in range(6)]

                # ==== phase 1a: pt, encode(x)
                with (
                    tc.tile_pool(name="ps1", bufs=3, space="PSUM") as ps_pool,
                    tc.tile_pool(name="wm", bufs=1) as wm_pool,
                ):
                    self.ps_pool = ps_pool
                    wm = self.load_weight_tiles(wm_pool, self.d_w["Wm"], D, D)
                    rl1 = tc.For_i(0, krep2, 1) if krep2 > 1 else None
                    if rl1 is not None:
                        rl1.__enter__()
                    for c in range(self.NCH):
                        cs = slice(c * CH, (c + 1) * CH)
                        for m in range(6):
                            ps = ps_pool.tile([128, CH], F32, tag="ps",
                                              name="ps")
                            for k in range(6):
                                nc.tensor.matmul(
                                    ps[:], wm[k][:, m * 128:(m + 1) * 128],
                                    xt[k][:, cs], start=(k == 0), stop=(k == 5))
                            nc.scalar.activation(pt[m][:, cs], ps[:],
                                                 AF.Identity,
                                                 bias=self.vcol("bm", m))
                    for c in range(self.NCH):
                        cs = slice(c * CH, (c + 1) * CH)
                        self.encode_ch([xt[k][:, cs] for k in range(6)],
                                       latx[:, cs])
                    if rl1 is not None:
                        rl1.__exit__(None, None, None)

                # ==== phase 1b: decode, recon, trans
                with (
                    tc.tile_pool(name="ps1b", bufs=3, space="PSUM") as ps_pool,
                    tc.tile_pool(name="wdec", bufs=1) as wdec_pool,
                ):
                    self.ps_pool = ps_pool
                    wd1 = self.load_weight_tiles(wdec_pool, self.d_w["Wd1"],
                                                 L, H)
                    wd2 = self.load_weight_tiles(wdec_pool, self.d_w["Wd2"],
                                                 H, H)
                    wd3 = self.load_weight_tiles(wdec_pool, self.d_w["Wd3"],
                                                 H, D)
                    rl2 = tc.For_i(0, krep2, 1) if krep2 > 1 else None
                    if rl2 is not None:
                        rl2.__enter__()
                    for c in range(self.NCH):
                        cs = slice(c * CH, (c + 1) * CH)
                        lat16 = self.tmp_pool.tile([128, CH], BF16, tag="h0",
                                                   name="lat16")
                        nc.vector.tensor_copy(lat16[:], latx[:, cs])
                        h1 = self.layer_ch([lat16[:]], wd1, H, bias="bd1",
                                           ln=("gd1", "bed1"), relu=True,
                                           out_tag="a")
                        h2 = self.layer_ch(h1, wd2, H, bias="bd2",
                                           ln=("gd2", "bed2"), relu=True,
                                           out_tag="h")
                        # recon = clip(mean((dec - x)^2), 0, 10), fused:
                        # r = (psum_dec + bd3) - x, per output tile
                        psr = ps_pool.tile([1, CH], F32, tag="st1", name="st1",
                                           bufs=2)
                        for m in range(6):
                            ps = ps_pool.tile([128, CH], F32, tag="ps",
                                              name="ps")
                            for k in range(4):
                                nc.tensor.matmul(
                                    ps[:], wd3[k][:, m * 128:(m + 1) * 128],
                                    h2[k], start=(k == 0), stop=(k == 3))
                            r = tmp_pool.tile([128, CH], F32, tag="sq",
                                              name="sq")
                            nc.vector.scalar_tensor_tensor(
                                r[:], ps[:], self.vcol("bd3", m), xt[m][:, cs],
                                ALU.add, ALU.subtract)
                            nc.vector.tensor_mul(r[:], r[:], r[:])
                            nc.tensor.matmul(psr[:], self.ones1[:, 0:1], r[:],
                                             start=(m == 0), stop=(m == 5))
                        recc = self.row_pool.tile([1, CH], F32, name="row",
                                                  tag="recc")
                        nc.vector.tensor_scalar(recc[:], psr[:],
                                                1.0 / D, 10.0, ALU.mult,
                                                ALU.min)
                        nc.sync.dma_start(self.d_rec[:, cs], recc[:])

                        # trans: dif chunk vs prev-token lat
                        dif = tmp_pool.tile([128, CH], F32, tag="dif",
                                            name="dif")
                        cst = c * CH
                        if c == 0:
                            nc.vector.tensor_sub(dif[:, 1:CH],
                                                 latx[:, cst + 1:cst + CH],
                                                 latx[:, cst:cst + CH - 1])
                            nc.vector.tensor_sub(dif[:, 0:1], latx[:, 0:1],
                                                 self.vcol("z0"))
                        else:
                            nc.vector.tensor_sub(
                                dif[:], latx[:, cst:cst + CH],
                                latx[:, cst - 1:cst + CH - 1])
                        for b in range(BL):
                            c0 = b * S
                            if c0 > 0 and cst <= c0 <= cst + CH - 1:
                                nc.vector.tensor_sub(
                                    dif[:, c0 - cst:c0 - cst + 1],
                                    latx[:, c0:c0 + 1], self.vcol("z0"))
                        trnc = self.row_pool.tile([1, CH], F32, name="row",
                                                  tag="trnc")
                        self.sumsq_chunk([dif[:]], trnc[:], 1.0 / L)
                        nc.sync.dma_start(self.d_trn[:, cs], trnc[:])
                    if rl2 is not None:
                        rl2.__exit__(None, None, None)

                xt_cm.__exit__(None, None, None)

                # ==== phase 2: recurrence (one ACT per step; pt added to
                # PSUM by an identity matmul; all m-groups in one PSUM bank)
                memw = big_pool.tile([128, 6 * NT], BF16, tag="memw",
                                     name="memw")
                memsb = [memw[:, k * NT:(k + 1) * NT] for k in range(6)]
                ptv = ptw[:].rearrange("p (m b s) -> p m b s", m=6, b=BL)
                memv = memw[:].rearrange("p (m b s) -> p m b s", m=6, b=BL)
                with (
                    tc.tile_pool(name="um", bufs=1) as um_pool,
                    tc.tile_pool(name="recps", bufs=2, space="PSUM") as rps,
                ):
                    um = self.load_weight_tiles(um_pool, self.d_w["Um"], D, D)
                    id_sb = um_pool.tile([128, 128], BF16, name="id_sb")
                    nc.sync.dma_start(id_sb[:], self.d_id[:, :])
                    nc.scalar.activation(memv[:, :, :, 0], ptv[:, :, :, 0],
                                         AF.Tanh)
                    import os as _os
                    krep = int(_os.environ.get("KREP", "1"))
                    rl = tc.For_i(0, krep, 1) if krep > 1 else None
                    if rl is not None:
                        rl.__enter__()
                    for j in range(1, S):
                        ps = rps.tile([128, 6 * BL], F32, tag="rps",
                                      name="rps")
                        psv = ps[:].rearrange("p (m b) -> p m b", m=6)
                        nc.tensor.matmul(ps[:], id_sb[:], ptv[:, :, :, j],
                                         start=True, stop=False)
                        for m in range(6):
                            for k in range(6):
                                last = (m == 5 and k == 5)
                                nc.tensor.matmul(
                                    ps[:, m * BL:(m + 1) * BL],
                                    um[k][:, m * 128:(m + 1) * 128],
                                    memv[:, k, :, j - 1],
                                    start=False, stop=last,
                                    skip_group_check=not last)
                        nc.scalar.activation(memv[:, :, :, j], psv[:],
                                             AF.Tanh)
                    if rl is not None:
                        rl.__exit__(None, None, None)

                pt_cm.__exit__(None, None, None)

                # ==== phase 3: encode(mem), ctx, combine
                with tc.tile_pool(name="ps3", bufs=3, space="PSUM") as ps_pool3:
                    self.ps_pool = ps_pool3
                    rl3 = tc.For_i(0, krep2, 1) if krep2 > 1 else None
                    if rl3 is not None:
                        rl3.__enter__()
                    for c in range(self.NCH):
                        cs = slice(c * CH, (c + 1) * CH)
                        self.encode_ch([memsb[k][:, cs] for k in range(6)],
                                       latm[:, cs])
                    for c in range(self.NCH):
                        cs = slice(c * CH, (c + 1) * CH)
                        nxc = self.row_pool.tile([1, CH], F32, name="row",
                                                 tag="nxc")
                        nmc = self.row_pool.tile([1, CH], F32, name="row",
                                                 tag="nmc")
                        dotc = self.row_pool.tile([1, CH], F32, name="row",
                                                  tag="dotc")
                        r1 = self.row_pool.tile([1, CH], F32, name="row",
                                                tag="r1")
                        self.sumsq_chunk([latx[:, cs]], nxc[:], 1.0,
                                         clip10=False)
                        self.sumsq_chunk([latm[:, cs]], nmc[:], 1.0,
                                         clip10=False)
                        self.sumsq_chunk([latx[:, cs]], dotc[:], 1.0,
                                         clip10=False, b_aps=[latm[:, cs]])
                        for rr in (nxc, nmc):
                            nc.scalar.activation(r1[:], rr[:], AF.Sqrt)
                            nc.vector.tensor_scalar_max(r1[:], r1[:],
                                                        NORM_EPS)
                            nc.vector.reciprocal(rr[:], r1[:])
                        nc.vector.tensor_mul(dotc[:], dotc[:], nxc[:])
                        nc.vector.tensor_mul(dotc[:], dotc[:], nmc[:])
                        # ctx = clip(1 - cos, 0, 10) (reuse nxc)
                        nc.vector.tensor_scalar(nxc[:], dotc[:], -1.0, 1.0,
                                                ALU.mult, ALU.add)
                        nc.vector.tensor_scalar(nxc[:], nxc[:], 0.0, 10.0,
                                                ALU.max, ALU.min)
                        recc = self.row_pool.tile([1, CH], F32, name="row",
                                                  tag="recc")
                        trnc = self.row_pool.tile([1, CH], F32, name="row",
                                                  tag="trnc")
                        nc.sync.dma_start(recc[:], self.d_rec[:, cs])
                        nc.sync.dma_start(trnc[:], self.d_trn[:, cs])
                        tokc = self.row_pool.tile([1, CH], F32, name="row",
                                                  tag="tokc")
                        nc.vector.scalar_tensor_tensor(
                            tokc[:], trnc[:], 0.3, recc[:], ALU.mult, ALU.add)
                        nc.vector.scalar_tensor_tensor(
                            tokc[:], nxc[:], 0.3, tokc[:], ALU.mult, ALU.add)
                        nc.sync.dma_start(self.d_out[:, cs], tokc[:])
                    if rl3 is not None:
                        rl3.__exit__(None, None, None)
        nc.compile()
        return nc


# ---------------------------------------------------------------- runner

_CACHE = {}


def _get_built(S, BL):
    key = (S, BL)
    if key not in _CACHE:
        kb = _KB(S, BL)
        kb.build()
        _CACHE[key] = kb
    return _CACHE[key]


def _host_inputs(kb, inputs):
    """Build the per-core in_maps from full inputs."""
    S, BL = kb.S, kb.BL
    w = {k: np.asarray(v, np.float32) for k, v in inputs.items()}
    Wvo = w["Wv"] @ w["Wo"]
    bvo = w["bv"] @ w["Wo"] + w["bo"]
    wd = dict(w)
    wd["Wvo"], wd["bvo"] = Wvo, bvo
    z0 = _encode_np(np.zeros((1, D), np.float32), wd)[0]

    vecs = _pack_cols(w["b1"], w["g1"], w["be1"], bvo,
                      w["b2"], w["g2"], w["be2"],
                      w["b3"], w["g3"], w["be3"],
                      w["b4"], w["g4"], w["be4"],
                      w["b5"], w["g5"], w["be5"],
                      w["bd1"], w["gd1"], w["bed1"],
                      w["bd2"], w["gd2"], w["bed2"],
                      w["bd3"], w["bm"], z0,
                      np.full(128, LN_EPS, np.float32))

    def b16(x):
        return np.ascontiguousarray(x.astype(ml_dtypes.bfloat16))

    ident = np.eye(128, dtype=np.float32)
    shared = dict(id16=b16(ident),
                  w116=b16(w["W1"]), wvo16=b16(Wvo), w216=b16(w["W2"]),
                  w316=b16(w["W3"]), w416=b16(w["W4"]), w516=b16(w["W5"]),
                  wd116=b16(w["Wd1"]), wd216=b16(w["Wd2"]),
                  wd316=b16(w["Wd3"]), wm16=b16(w["Wm"]), um16=b16(w["Um"]),
                  vecs=vecs)

    seqs = np.asarray(inputs["sequences"], np.float32)
    in_maps = []
    for c in range(NCORES):
        xs = seqs[c * BL:(c + 1) * BL, :S, :]           # [BL, S, D]
        xt = b16(xs.reshape(BL * S, D).T)
        m = dict(shared)
        m["xt"] = xt
        in_maps.append(m)
    return in_maps


def _l2_term(inputs):
    names = ["W1", "b1", "g1", "be1", "Wv", "bv", "Wo", "bo", "W2", "b2", "g2",
             "be2", "W3", "b3", "g3", "be3", "W4", "b4", "g4", "be4", "W5",
             "b5", "g5", "be5", "Wd1", "bd1", "gd1", "bed1", "Wd2", "bd2",
             "gd2", "bed2", "Wd3", "bd3", "Wm", "Um", "bm"]
    l2 = sum(np.linalg.norm(np.asarray(inputs[n], np.float64)) for n in names)
    return float(np.clip(l2, 0.0, 10.0))


def _combine(kb, res, inputs):
    tok = np.concatenate([res.results[c]["tok_loss"].reshape(-1)
                          for c in range(NCORES)])
    l2 = _l2_term(inputs)
    per_tok = np.clip(tok.astype(np.float64) + 1e-4 * l2, 0.0, 100.0)
    nb = kb.BL * NCORES
    return np.float32(per_tok.sum() / nb)


def kernel(**inputs):
    seqs = np.asarray(inputs["sequences"])
    S = seqs.shape[1]
    BL = seqs.shape[0] // NCORES
    kb = _get_built(S, BL)
    in_maps = _host_inputs(kb, inputs)
    res = run_bass_kernel_spmd(kb.nc, in_maps, list(range(NCORES)))
    return _combine(kb, res, inputs)



# revision 31
# speedup vs baseline: 1.1123x; 1.1123x over previous
"""Trainium2 Bass kernel for nn_EmotionalEmbeddingSpace.

Sharding: data-parallel over batch B=16 across 8 cores (2 sequences/core).
Layout on device: transposed - features on partitions, tokens on the free dim.

Per core (BL=2 sequences, NTOK=BL*S tokens):
  pt   = Wm^T @ x^T + bm                 (bulk matmul)
  mem_j = tanh(pt_j + Um^T @ mem_{j-1})  (serial; split PSUM groups so tanh
                                          hides behind the LDWEIGHTS stream)
  latx = encode(x), latm = encode(mem)   (MLP chains, breadth-first across
                                          4 token chunks so cross-engine
                                          latency hides behind other chunks)
  recon/trans/ctx per-token losses -> tok_loss[NTOK] -> host adds l2 term.

Matmul inputs bf16 (f32 PSUM); LN stats from bf16 y/sq via dual ACT
evacuation (Identity + Square); row math f32.
"""

import sys

sys.path.insert(0, "/opt/trn_rl_repo")

import numpy as np
import ml_dtypes

import concourse.bass as bass
import concourse.bacc as bacc
import concourse.mybir as mybir
import concourse.tile as tile
from concourse.bass_utils import run_bass_kernel_spmd

F32 = mybir.dt.float32
BF16 = mybir.dt.bfloat16
AF = mybir.ActivationFunctionType
ALU = mybir.AluOpType

B, S_FULL, D, H, L = 16, 1024, 768, 512, 128
NCORES = 8
LN_EPS = 1e-5
NORM_EPS = 1e-8


# ---------------------------------------------------------------- host prep

def _pack_cols(*vecs):
    cols = []
    for v in vecs:
        v = np.asarray(v, np.float32).reshape(-1, 128)
        cols.append(v.T)
    return np.ascontiguousarray(np.concatenate(cols, axis=1))


def _ln_np(x, g, b, eps=LN_EPS):
    m = x.mean(-1, keepdims=True)
    v = ((x - m) ** 2).mean(-1, keepdims=True)
    return (x - m) / np.sqrt(v + eps) * g + b


def _encode_np(t, w):
    h = np.maximum(_ln_np(t @ w["W1"] + w["b1"], w["g1"], w["be1"]), 0)
    a = h @ w["Wvo"] + w["bvo"]
    g = np.maximum(_ln_np(a @ w["W2"] + w["b2"], w["g2"], w["be2"]), 0)
    zl = _ln_np(g @ w["W3"] + w["b3"], w["g3"], w["be3"])
    e = np.maximum(_ln_np(zl @ w["W4"] + w["b4"], w["g4"], w["be4"]), 0)
    return _ln_np(e @ w["W5"] + w["b5"], w["g5"], w["be5"])


# ---------------------------------------------------------------- builder

class _KB:
    def __init__(self, S=S_FULL, BL=B // NCORES):
        self.S, self.BL = S, BL
        self.NTOK = S * BL
        self.CH = min(512, self.NTOK)
        self.NCH = self.NTOK // self.CH
        self.nc = bacc.Bacc("TRN2", target_bir_lowering=False, debug=False,
                            num_devices=NCORES)
        self.vec_map = {}
        self._vec_cols = 0

    def _reg_vec(self, name, ntiles):
        self.vec_map[name] = (self._vec_cols, ntiles)
        self._vec_cols += ntiles

    def declare(self):
        nc = self.nc
        NT = self.NTOK
        self.d_xt = nc.dram_tensor("xt", [D, NT], BF16, kind="ExternalInput")
        wshapes = dict(W1=(D, H), Wvo=(H, H), W2=(H, H), W3=(H, L), W4=(L, H),
                       W5=(H, L), Wd1=(L, H), Wd2=(H, H), Wd3=(H, D),
                       Wm=(D, D), Um=(D, D))
        self.d_w = {k: nc.dram_tensor(k.lower() + "16", list(v), BF16,
                                      kind="ExternalInput")
                    for k, v in wshapes.items()}
        for nm, n in [("b1", 4), ("g1", 4), ("be1", 4), ("bvo", 4),
                      ("b2", 4), ("g2", 4), ("be2", 4),
                      ("b3", 1), ("g3", 1), ("be3", 1),
                      ("b4", 4), ("g4", 4), ("be4", 4),
                      ("b5", 1), ("g5", 1), ("be5", 1),
                      ("bd1", 4), ("gd1", 4), ("bed1", 4),
                      ("bd2", 4), ("gd2", 4), ("bed2", 4),
                      ("bd3", 6), ("bm", 6), ("z0", 1), ("lneps", 1)]:
            self._reg_vec(nm, n)
        self.d_vecs = nc.dram_tensor("vecs", [128, self._vec_cols], F32,
                                     kind="ExternalInput")
        self.d_id = nc.dram_tensor("id16", [128, 128], BF16,
                                   kind="ExternalInput")
        self.d_out = nc.dram_tensor("tok_loss", [1, NT], F32,
                                    kind="ExternalOutput")

    def vcol(self, name, t=0):
        s, n = self.vec_map[name]
        assert t < n
        return self.vecs_sb[:, s + t:s + t + 1]

    # ---- helpers --------------------------------------------------------
    def load_weight_tiles(self, pool, dram, K, M):
        nc = self.nc
        tiles = []
        for k in range(K // 128):
            t = pool.tile([128, M], BF16, tag=f"w_{dram.name}_{k}",
                          name=f"w_{dram.name}_{k}")
            nc.sync.dma_start(t[:], dram[k * 128:(k + 1) * 128, :])
            tiles.append(t)
        return tiles

    def layer_bf(self, chs_in, w_tiles, M_out, *, bias, ln=None, relu=False,
                 out_slot=0, out_override=None, out_dtype=BF16):
        """Breadth-first layer over len(chs_in) chunks.

        chs_in: {c: [in_aps]} (bf16 [128, CH]).  Returns {c: [out_aps]}.
        out_override: {c: ap} for single-tile output written to a big tensor.
        """
        nc, CH = self.nc, self.CH
        cs_list = sorted(chs_in.keys())
        n_k = len(chs_in[cs_list[0]])
        n_m = M_out // 128
        tp = self.tmp_pool
        outs = {}
        for c in cs_list:
            if out_override is not None:
                outs[c] = [out_override[c]]
            else:
                outs[c] = [tp.tile([128, CH], out_dtype,
                                   tag=f"t{out_slot}m{m}c{c}",
                                   name=f"t{out_slot}m{m}c{c}")[:]
                           for m in range(n_m)]
        ys = {c: [] for c in cs_list}
        sts = {}
        if ln is not None:
            for c in cs_list:
                sts[c] = self.sp.tile([33, CH], F32, tag=f"st{c}",
                                      name=f"st{c}")
        # main matmuls + evacuation, m-outer / c-inner
        for m in range(n_m):
            for c in cs_list:
                ps = self.pp.tile([128, CH], F32, tag=f"ps{c}",
                                  name=f"ps{c}")
                for k in range(n_k):
                    nc.tensor.matmul(ps[:],
                                     w_tiles[k][:, m * 128:(m + 1) * 128],
                                     chs_in[c][k], start=(k == 0),
                                     stop=(k == n_k - 1))
                if ln is None:
                    nc.scalar.activation(outs[c][m], ps[:],
                                         AF.Relu if relu else AF.Identity,
                                         bias=self.vcol(bias, m))
                else:
                    y = tp.tile([128, CH], BF16, tag=f"y{m}c{c}",
                                name=f"y{m}c{c}")
                    nc.scalar.activation(y[:], ps[:], AF.Identity,
                                         bias=self.vcol(bias, m))
                    sq = tp.tile([128, CH], BF16, tag=f"ub{c}",
                                 name=f"sq{c}")
                    nc.scalar.activation(sq[:], ps[:], AF.Square,
                                         bias=self.vcol(bias, m))
                    ys[c].append(y)
                    # stats: row0 += sum(y), row1 += sum(sq)
                    st = sts[c]
                    nc.tensor.matmul(st[:], self.onesA[:, 0:33], y[:],
                                     start=(m == 0), stop=False,
                                     skip_group_check=(m != 0))
                    nc.tensor.matmul(st[:], self.onesB[:, 0:33], sq[:],
                                     start=False, stop=(m == n_m - 1),
                                     skip_group_check=(m != n_m - 1))
        if ln is None:
            return outs
        g_nm, be_nm = ln
        inv_f = 1.0 / M_out
        # row math per chunk: all compute rows live on partition 0,
        # packed along the free dim (seg0 = mean->mr, seg1 = var->rstd);
        # the Sigma(y^2) psum row is read at partition 32 (quadrant start).
        rbs = {}
        for c in cs_list:
            st = sts[c]
            r = self.row_pool.tile([1, 2 * CH], F32, tag=f"rows{c}",
                                   name=f"rows{c}")
            s0 = r[0:1, 0:CH]
            s1 = r[0:1, CH:2 * CH]
            nc.vector.tensor_scalar_mul(s0, st[0:1, :], inv_f)
            nc.vector.scalar_tensor_tensor(s1, s0, -1.0, s0,
                                           ALU.mult, ALU.mult)
            nc.vector.scalar_tensor_tensor(s1, st[32:33, :], inv_f, s1,
                                           ALU.mult, ALU.add)
            nc.scalar.activation(s1, s1, AF.Sqrt,
                                 bias=self.vcol("lneps")[0:1])
            nc.vector.reciprocal(s1, s1)
            nc.vector.tensor_mul(s0, s0, s1)
            rrA = self.row_pool.tile([1, CH], BF16, tag=f"rrA{c}",
                                     name=f"rrA{c}")
            rrB = self.row_pool.tile([1, CH], BF16, tag=f"rrB{c}",
                                     name=f"rrB{c}")
            nc.vector.tensor_copy(rrA[:], s1)
            nc.vector.tensor_copy(rrB[:], s0)
            rb = self.tmp_pool.tile([128, CH], BF16, tag=f"rb{c}",
                                    name=f"rb{c}")
            mrb = self.tmp_pool.tile([128, CH], BF16, tag=f"mrb{c}",
                                     name=f"mrb{c}")
            nc.gpsimd.partition_broadcast(rb[:], rrA[:])
            nc.gpsimd.partition_broadcast(mrb[:], rrB[:])
            rbs[c] = (rb, mrb)
        # apply
        for m in range(n_m):
            for c in cs_list:
                rb, mrb = rbs[c]
                u = self.tmp_pool.tile([128, CH], BF16, tag=f"ub{c}",
                                       name=f"ub{c}")
                nc.vector.tensor_mul(u[:], ys[c][m][:], rb[:])
                nc.vector.tensor_sub(u[:], u[:], mrb[:])
                nc.scalar.activation(outs[c][m], u[:],
                                     AF.Relu if relu else AF.Identity,
                                     bias=self.vcol(be_nm, m),
                                     scale=self.vcol(g_nm, m))
        return outs

    def encode_bf(self, chs_in, out_override, out_dtype):
        h = self.layer_bf(chs_in, self.w_sb["W1"], H, bias="b1",
                          ln=("g1", "be1"), relu=True, out_slot=0)
        a = self.layer_bf(h, self.w_sb["Wvo"], H, bias="bvo", out_slot=1)
        g = self.layer_bf(a, self.w_sb["W2"], H, bias="b2",
                          ln=("g2", "be2"), relu=True, out_slot=0)
        zl = self.layer_bf(g, self.w_sb["W3"], L, bias="b3",
                           ln=("g3", "be3"), out_slot=1)
        e = self.layer_bf(zl, self.w_sb["W4"], H, bias="b4",
                          ln=("g4", "be4"), relu=True, out_slot=0)
        self.layer_bf(e, self.w_sb["W5"], L, bias="b5", ln=("g5", "be5"),
                      out_override=out_override, out_dtype=out_dtype)

    # ---- main build -----------------------------------------------------
    def build(self):
        nc = self.nc
        NT, CH, S, BL = self.NTOK, self.CH, self.S, self.BL
        NCH = self.NCH
        self.declare()
        import os as _os
        skip_rec = _os.environ.get("SKIP_REC") == "1"
        with tile.TileContext(nc) as tc:
            with (
                tc.tile_pool(name="const", bufs=1) as const_pool,
                tc.tile_pool(name="wenc", bufs=1) as wenc_pool,
                tc.tile_pool(name="big", bufs=1) as big_pool,
                tc.tile_pool(name="tmp", bufs=1) as tmp_pool,
                tc.tile_pool(name="rows", bufs=1) as row_pool,
            ):
                self.tmp_pool, self.row_pool = tmp_pool, row_pool

                # constants
                self.onesA = const_pool.tile([128, 33], BF16, name="onesA")
                self.onesB = const_pool.tile([128, 33], BF16, name="onesB")
                nc.vector.memset(self.onesA[:], 0.0)
                nc.vector.memset(self.onesA[:, 0:1], 1.0)
                nc.vector.memset(self.onesB[:], 0.0)
                nc.vector.memset(self.onesB[:, 32:33], 1.0)
                self.vecs_sb = const_pool.tile([128, self._vec_cols], F32)
                nc.sync.dma_start(self.vecs_sb[:], self.d_vecs[:, :])
                self.z016 = const_pool.tile([128, 1], BF16, name="z016")
                nc.vector.tensor_copy(self.z016[:], self.vcol("z0"))

                self.w_sb = {}
                for k, (K, M) in dict(W1=(D, H), Wvo=(H, H), W2=(H, H),
                                      W3=(H, L), W4=(L, H), W5=(H, L)).items():
                    self.w_sb[k] = self.load_weight_tiles(wenc_pool,
                                                          self.d_w[k], K, M)

                latx = big_pool.tile([128, NT], BF16, tag="latx", name="latx")
                latm = big_pool.tile([128, NT], BF16, tag="latm", name="latm")
                lrow = {c: row_pool.tile([1, 2 * CH], F32, tag=f"lrow{c}",
                                         name=f"lrow{c}")
                        for c in range(NCH)}

                xt_cm = tc.tile_pool(name="xtp", bufs=1)
                xt_pool = xt_cm.__enter__()
                xt = [xt_pool.tile([128, NT], BF16, tag=f"xt{k}",
                                   name=f"xt{k}") for k in range(6)]
                for k in range(6):
                    nc.sync.dma_start(xt[k][:],
                                      self.d_xt[k * 128:(k + 1) * 128, :])
                pt_cm = tc.tile_pool(name="ptp", bufs=1)
                pt_pool = pt_cm.__enter__()
                ptw = pt_pool.tile([128, 6 * NT], BF16, tag="ptw",
                                   name="ptw")
                pt = [ptw[:, m * NT:(m + 1) * NT] for m in range(6)]

                # ==== phase 0: pt = Wm^T x + bm  (breadth-first)
                with (
                    tc.tile_pool(name="ps0", bufs=1, space="PSUM") as pp0,
                    tc.tile_pool(name="wm", bufs=1) as wm_pool,
                ):
                    wm = self.load_weight_tiles(wm_pool, self.d_w["Wm"], D, D)
                    for m in range(6):
                        for c in range(NCH):
                            cs = slice(c * CH, (c + 1) * CH)
                            ps = pp0.tile([128, CH], F32, tag=f"p{c}",
                                          name=f"p{c}")
                            for k in range(6):
                                nc.tensor.matmul(
                                    ps[:], wm[k][:, m * 128:(m + 1) * 128],
                                    xt[k][:, cs], start=(k == 0),
                                    stop=(k == 5))
                            nc.scalar.activation(pt[m][:, cs], ps[:],
                                                 AF.Identity,
                                                 bias=self.vcol("bm", m))

                # ==== phase 1: recurrence (split PSUM groups)
                memw = big_pool.tile([128, 6 * NT], BF16, tag="memw",
                                     name="memw")
                memsb = [memw[:, k * NT:(k + 1) * NT] for k in range(6)]
                ptv = ptw[:].rearrange("p (m b s) -> p m b s", m=6, b=BL)
                memv = memw[:].rearrange("p (m b s) -> p m b s", m=6, b=BL)
                with (
                    tc.tile_pool(name="um", bufs=1) as um_pool,
                    tc.tile_pool(name="recps", bufs=1, space="PSUM") as rps,
                ):
                    um = self.load_weight_tiles(um_pool, self.d_w["Um"], D, D)
                    id_sb = um_pool.tile([128, 128], BF16, name="id_sb")
                    nc.sync.dma_start(id_sb[:], self.d_id[:, :])
                    G = 3 * BL
                    if skip_rec:
                        nc.vector.memset(memw[:], 0.1)
                    nc.scalar.activation(memv[:, 0:3, :, 0],
                                         ptv[:, 0:3, :, 0], AF.Tanh)
                    nc.scalar.activation(memv[:, 3:6, :, 0],
                                         ptv[:, 3:6, :, 0], AF.Tanh)
                    nsteps = 2 if skip_rec else S
                    for j in range(1, nsteps):
                        pss = []
                        for g in range(2):
                            ps = rps.tile([128, G], F32, tag=f"rps{g}",
                                          name=f"rps{g}", bufs=2,
                                          padded_shape=[128, 512])
                            nc.tensor.matmul(ps[:], id_sb[:],
                                             ptv[:, 3 * g:3 * g + 3, :, j],
                                             start=True, stop=False)
                            pss.append(ps)
                        for g in range(2):
                            ps = pss[g]
                            for k in range(6):
                                for mi in range(3):
                                    m = 3 * g + mi
                                    last = (k == 5 and mi == 2)
                                    nc.tensor.matmul(
                                        ps[:, mi * BL:(mi + 1) * BL],
                                        um[k][:, m * 128:(m + 1) * 128],
                                        memv[:, k, :, j - 1],
                                        start=False, stop=last,
                                        skip_group_check=not last)
                            psv = ps[:].rearrange("p (m b) -> p m b", m=3)
                            nc.scalar.activation(
                                memv[:, 3 * g:3 * g + 3, :, j], psv[:],
                                AF.Tanh)
                pt_cm.__exit__(None, None, None)

                # ==== MLP phases (breadth-first over chunks)
                mlp_ps = tc.tile_pool(name="mps", bufs=1, space="PSUM")
                self.pp = mlp_ps.__enter__()
                mlp_sp = tc.tile_pool(name="msp", bufs=1, space="PSUM")
                self.sp = mlp_sp.__enter__()

                allc = list(range(NCH))
                xt_chs = {c: [xt[k][:, c * CH:(c + 1) * CH] for k in range(6)]
                          for c in allc}
                lat_ov = {c: latx[:, c * CH:(c + 1) * CH] for c in allc}

                # phase 2: encode(x) -> latx (bf16)
                self.encode_bf(xt_chs, lat_ov, BF16)

                # phase 3: decode + recon + trans
                with tc.tile_pool(name="wdec", bufs=1) as wdec_pool:
                    wd1 = self.load_weight_tiles(wdec_pool, self.d_w["Wd1"],
                                                 L, H)
                    wd2 = self.load_weight_tiles(wdec_pool, self.d_w["Wd2"],
                                                 H, H)
                    wd3 = self.load_weight_tiles(wdec_pool, self.d_w["Wd3"],
                                                 H, D)
                    lat16 = {c: [latx[:, c * CH:(c + 1) * CH]]
                             for c in allc}
                    h1 = self.layer_bf(lat16, wd1, H, bias="bd1",
                                       ln=("gd1", "bed1"), relu=True,
                                       out_slot=0)
                    h2 = self.layer_bf(h1, wd2, H, bias="bd2",
                                       ln=("gd2", "bed2"), relu=True,
                                       out_slot=1)
                    # Wd3 + recon: row0 of lrow[c]
                    rsts = {c: self.sp.tile([33, CH], F32, tag=f"st{c}",
                                            name=f"st{c}") for c in allc}
                    for m in range(6):
                        for c in allc:
                            cs = slice(c * CH, (c + 1) * CH)
                            ps = self.pp.tile([128, CH], F32, tag=f"ps{c}",
                                              name=f"ps{c}")
                            for k in range(4):
                                nc.tensor.matmul(
                                    ps[:], wd3[k][:, m * 128:(m + 1) * 128],
                                    h2[c][k], start=(k == 0), stop=(k == 3))
                            r = tmp_pool.tile([128, CH], BF16,
                                              tag=f"ub{c}", name=f"rr{c}")
                            nc.vector.scalar_tensor_tensor(
                                r[:], ps[:], self.vcol("bd3", m),
                                xt[m][:, cs], ALU.add, ALU.subtract)
                            nc.vector.tensor_mul(r[:], r[:], r[:])
                            st = rsts[c]
                            nc.tensor.matmul(st[:], self.onesA[:, 0:33],
                                             r[:],
                                             start=(m == 0), stop=False,
                                             skip_group_check=(m != 0))
                            if m == 5:
                                # trans into row1: dif of latx vs prev token
                                dif = tmp_pool.tile([128, CH], BF16,
                                                    tag=f"rb{c}",
                                                    name=f"dif{c}")
                                cst = c * CH
                                if cst == 0:
                                    nc.vector.tensor_sub(
                                        dif[:, 1:CH],
                                        latx[:, cst + 1:cst + CH],
                                        latx[:, cst:cst + CH - 1])
                                    nc.vector.tensor_sub(dif[:, 0:1],
                                                         latx[:, 0:1],
                                                         self.z016[:])
                                else:
                                    nc.vector.tensor_sub(
                                        dif[:], latx[:, cst:cst + CH],
                                        latx[:, cst - 1:cst + CH - 1])
                                for b in range(BL):
                                    c0 = b * S
                                    if c0 > 0 and cst <= c0 <= cst + CH - 1:
                                        nc.vector.tensor_sub(
                                            dif[:, c0 - cst:c0 - cst + 1],
                                            latx[:, c0:c0 + 1],
                                            self.z016[:])
                                dif2 = tmp_pool.tile([128, CH], BF16,
                                                     tag=f"ub{c}",
                                                     name=f"dif2{c}")
                                nc.vector.tensor_mul(dif2[:], dif[:], dif[:])
                                nc.tensor.matmul(st[:], self.onesB[:, 0:33],
                                                 dif2[:], start=False,
                                                 stop=True)
                    for c in allc:
                        st = rsts[c]
                        # lrow[c]: recon/D @ seg0, trans/L @ seg1
                        nc.vector.tensor_scalar(lrow[c][0:1, 0:CH],
                                                st[0:1, :],
                                                1.0 / D, 10.0, ALU.mult,
                                                ALU.min)
                        nc.vector.tensor_scalar(lrow[c][0:1, CH:2 * CH],
                                                st[32:33, :],
                                                1.0 / L, 10.0, ALU.mult,
                                                ALU.min)
                xt_cm.__exit__(None, None, None)

                # phase 4: encode(mem) -> latm (bf16)
                mem_chs = {c: [memsb[k][:, c * CH:(c + 1) * CH]
                               for k in range(6)] for c in allc}
                latm_ov = {c: latm[:, c * CH:(c + 1) * CH] for c in allc}
                self.encode_bf(mem_chs, latm_ov, BF16)

                # phase 5: ctx + combine (rows on partition 0, free-dim
                # segments; Sigma rows read from psum at partitions 0/32)
                csts = {c: self.sp.tile([33, CH], F32, tag=f"st{c}",
                                        name=f"st{c}") for c in allc}
                rowsd = {}
                for c in allc:
                    cs = slice(c * CH, (c + 1) * CH)
                    st = csts[c]
                    u = tmp_pool.tile([128, CH], BF16, tag=f"y0c{c}",
                                      name=f"cu{c}")
                    nc.vector.tensor_mul(u[:], latx[:, cs], latx[:, cs])
                    nc.tensor.matmul(st[:], self.onesA[:, 0:33], u[:],
                                     start=True, stop=False)
                    u2 = tmp_pool.tile([128, CH], BF16, tag=f"y1c{c}",
                                       name=f"cu2{c}")
                    nc.vector.tensor_mul(u2[:], latm[:, cs], latm[:, cs])
                    nc.tensor.matmul(st[:], self.onesB[:, 0:33], u2[:],
                                     start=False, stop=True)
                for c in allc:
                    st = csts[c]
                    # rows: seg0 = 1/max(sqrt(nx),eps) * later terms,
                    # seg1 = 1/max(sqrt(nm),eps)
                    r = self.row_pool.tile([1, 2 * CH], F32, tag=f"rows{c}",
                                           name=f"rows{c}")
                    rowsd[c] = r
                    s0 = r[0:1, 0:CH]
                    s1 = r[0:1, CH:2 * CH]
                    nc.scalar.activation(s0, st[0:1, :], AF.Sqrt)
                    nc.scalar.activation(s1, st[32:33, :], AF.Sqrt)
                    nc.vector.tensor_scalar_max(r[0:1, :], r[0:1, :],
                                                NORM_EPS)
                    nc.vector.reciprocal(r[0:1, :], r[0:1, :])
                csts2 = {c: self.sp.tile([33, CH], F32, tag=f"st{c}",
                                         name=f"st{c}") for c in allc}
                for c in allc:
                    cs = slice(c * CH, (c + 1) * CH)
                    u3 = tmp_pool.tile([128, CH], BF16, tag=f"y2c{c}",
                                       name=f"cu3{c}")
                    nc.vector.tensor_mul(u3[:], latx[:, cs], latm[:, cs])
                    nc.tensor.matmul(csts2[c][:], self.onesA[:, 0:33], u3[:],
                                     start=True, stop=True)
                for c in allc:
                    cs = slice(c * CH, (c + 1) * CH)
                    r = rowsd[c]
                    s0 = r[0:1, 0:CH]
                    s1 = r[0:1, CH:2 * CH]
                    # s0 = cos = dot * rx * rm
                    nc.vector.tensor_mul(s0, csts2[c][0:1, :], s0)
                    nc.vector.tensor_mul(s0, s0, s1)
                    # s0 = clip(1 - cos, 0, 10)
                    nc.vector.tensor_scalar(s0, s0, -1.0, 1.0,
                                            ALU.mult, ALU.add)
                    nc.vector.tensor_scalar(s0, s0, 0.0, 10.0,
                                            ALU.max, ALU.min)
                    # s1 = recon + 0.3*trans + 0.3*ctx
                    nc.vector.scalar_tensor_tensor(
                        s1, lrow[c][0:1, CH:2 * CH], 0.3,
                        lrow[c][0:1, 0:CH], ALU.mult, ALU.add)
                    nc.vector.scalar_tensor_tensor(
                        s1, s0, 0.3, s1, ALU.mult, ALU.add)
                    nc.sync.dma_start(self.d_out[:, cs], s1)

                mlp_sp.__exit__(None, None, None)
                mlp_ps.__exit__(None, None, None)
        nc.compile()
        return nc

# ---------------------------------------------------------------- runner

_CACHE = {}


def _get_built(S, BL):
    key = (S, BL)
    if key not in _CACHE:
        kb = _KB(S, BL)
        kb.build()
        _CACHE[key] = kb
    return _CACHE[key]


def _host_inputs(kb, inputs):
    S, BL = kb.S, kb.BL
    w = {k: np.asarray(v, np.float32) for k, v in inputs.items()}
    Wvo = w["Wv"] @ w["Wo"]
    bvo = w["bv"] @ w["Wo"] + w["bo"]
    wd = dict(w)
    wd["Wvo"], wd["bvo"] = Wvo, bvo
    z0 = _encode_np(np.zeros((1, D), np.float32), wd)[0]

    vecs = _pack_cols(w["b1"], w["g1"], w["be1"], bvo,
                      w["b2"], w["g2"], w["be2"],
                      w["b3"], w["g3"], w["be3"],
                      w["b4"], w["g4"], w["be4"],
                      w["b5"], w["g5"], w["be5"],
                      w["bd1"], w["gd1"], w["bed1"],
                      w["bd2"], w["gd2"], w["bed2"],
                      w["bd3"], w["bm"], z0,
                      np.full(128, LN_EPS, np.float32))

    def b16(x):
        return np.ascontiguousarray(x.astype(ml_dtypes.bfloat16))

    ident = np.eye(128, dtype=np.float32)
    shared = dict(id16=b16(ident),
                  w116=b16(w["W1"]), wvo16=b16(Wvo), w216=b16(w["W2"]),
                  w316=b16(w["W3"]), w416=b16(w["W4"]), w516=b16(w["W5"]),
                  wd116=b16(w["Wd1"]), wd216=b16(w["Wd2"]),
                  wd316=b16(w["Wd3"]), wm16=b16(w["Wm"]), um16=b16(w["Um"]),
                  vecs=vecs)

    seqs = np.asarray(inputs["sequences"], np.float32)
    in_maps = []
    for c in range(NCORES):
        xs = seqs[c * BL:(c + 1) * BL, :S, :]
        xt = b16(xs.reshape(BL * S, D).T)
        m = dict(shared)
        m["xt"] = xt
        in_maps.append(m)
    return in_maps


def _l2_term(inputs):
    names = ["W1", "b1", "g1", "be1", "Wv", "bv", "Wo", "bo", "W2", "b2", "g2",
             "be2", "W3", "b3", "g3", "be3", "W4", "b4", "g4", "be4", "W5",
             "b5", "g5", "be5", "Wd1", "bd1", "gd1", "bed1", "Wd2", "bd2",
             "gd2", "bed2", "Wd3", "bd3", "Wm", "Um", "bm"]
    l2 = sum(np.linalg.norm(np.asarray(inputs[n], np.float64)) for n in names)
    return float(np.clip(l2, 0.0, 10.0))


def _combine(kb, res, inputs):
    tok = np.concatenate([res.results[c]["tok_loss"].reshape(-1)
                          for c in range(NCORES)])
    l2 = _l2_term(inputs)
    per_tok = np.clip(tok.astype(np.float64) + 1e-4 * l2, 0.0, 100.0)
    nb = kb.BL * NCORES
    return np.float32(per_tok.sum() / nb)


def kernel(**inputs):
    seqs = np.asarray(inputs["sequences"])
    S = seqs.shape[1]
    BL = seqs.shape[0] // NCORES
    kb = _get_built(S, BL)
    in_maps = _host_inputs(kb, inputs)
    res = run_bass_kernel_spmd(kb.nc, in_maps, list(range(NCORES)))
    return _combine(kb, res, inputs)


# revision 33
# speedup vs baseline: 2.3756x; 2.1358x over previous
"""Trainium2 Bass kernel for nn_EmotionalEmbeddingSpace.

Sharding: data-parallel over batch B=16 across 8 cores (2 sequences/core).
Layout on device: transposed - features on partitions, tokens on the free dim.

Per core (BL=2 sequences, NTOK=BL*S tokens):
  pt   = Wm^T @ x^T + bm                 (bulk matmul)
  mem_j = tanh(pt_j + Um^T @ mem_{j-1})  (serial; split PSUM groups so tanh
                                          hides behind the LDWEIGHTS stream)
  latx = encode(x), latm = encode(mem)   (MLP chains, breadth-first across
                                          4 token chunks so cross-engine
                                          latency hides behind other chunks)
  recon/trans/ctx per-token losses -> tok_loss[NTOK] -> host adds l2 term.

Matmul inputs bf16 (f32 PSUM); LN stats from bf16 y/sq via dual ACT
evacuation (Identity + Square); row math f32.
"""

import sys

sys.path.insert(0, "/opt/trn_rl_repo")

import numpy as np
import ml_dtypes

import concourse.bass as bass
import concourse.bacc as bacc
import concourse.mybir as mybir
import concourse.tile as tile
from concourse.bass_utils import run_bass_kernel_spmd

F32 = mybir.dt.float32
BF16 = mybir.dt.bfloat16
AF = mybir.ActivationFunctionType
ALU = mybir.AluOpType

B, S_FULL, D, H, L = 16, 1024, 768, 512, 128
NCORES = 8
LN_EPS = 1e-5
NORM_EPS = 1e-8


# ---------------------------------------------------------------- host prep

def _pack_cols(*vecs):
    cols = []
    for v in vecs:
        v = np.asarray(v, np.float32).reshape(-1, 128)
        cols.append(v.T)
    return np.ascontiguousarray(np.concatenate(cols, axis=1))


def _ln_np(x, g, b, eps=LN_EPS):
    m = x.mean(-1, keepdims=True)
    v = ((x - m) ** 2).mean(-1, keepdims=True)
    return (x - m) / np.sqrt(v + eps) * g + b


def _encode_np(t, w):
    h = np.maximum(_ln_np(t @ w["W1"] + w["b1"], w["g1"], w["be1"]), 0)
    a = h @ w["Wvo"] + w["bvo"]
    g = np.maximum(_ln_np(a @ w["W2"] + w["b2"], w["g2"], w["be2"]), 0)
    zl = _ln_np(g @ w["W3"] + w["b3"], w["g3"], w["be3"])
    e = np.maximum(_ln_np(zl @ w["W4"] + w["b4"], w["g4"], w["be4"]), 0)
    return _ln_np(e @ w["W5"] + w["b5"], w["g5"], w["be5"])


# ---------------------------------------------------------------- builder

class _KB:
    def __init__(self, S=S_FULL, BL=B // NCORES):
        self.S, self.BL = S, BL
        self.NTOK = S * BL
        self.CH = min(512, self.NTOK)
        self.NCH = self.NTOK // self.CH
        self.nc = bacc.Bacc("TRN2", target_bir_lowering=False, debug=False,
                            num_devices=NCORES)
        self.vec_map = {}
        self._vec_cols = 0

    def _reg_vec(self, name, ntiles):
        self.vec_map[name] = (self._vec_cols, ntiles)
        self._vec_cols += ntiles

    WSHAPES = dict(W1=(D, H), Wvo=(H, H), W2=(H, H), W3=(H, L), W4=(L, H),
                   W5=(H, L), Wd1=(L, H), Wd2=(H, H), Wd3=(H, D),
                   Wm=(D, D), Um=(D, D))

    def blob_layout(self):
        """Column layout of the single bf16 input blob: xt tiles first,
        then each weight as K//128 row-tiles of M columns, then identity."""
        NT = self.NTOK
        entries = [("xt", 6, NT)]
        for k, (K, M) in self.WSHAPES.items():
            entries.append((k, K // 128, M))
        entries.append(("id", 1, 128))
        off = {}
        pos = 0
        for name, ntiles, M in entries:
            off[name] = (pos, ntiles, M)
            pos += ntiles * M
        return off, pos

    def declare(self):
        nc = self.nc
        NT = self.NTOK
        self.blob_off, nblob = self.blob_layout()
        self.d_blob = nc.dram_tensor("blob16", [128, nblob], BF16,
                                     kind="ExternalInput")
        for nm, n in [("b1", 4), ("g1", 4), ("be1", 4), ("bvo", 4),
                      ("b2", 4), ("g2", 4), ("be2", 4),
                      ("b3", 1), ("g3", 1), ("be3", 1),
                      ("b4", 4), ("g4", 4), ("be4", 4),
                      ("b5", 1), ("g5", 1), ("be5", 1),
                      ("bd1", 4), ("gd1", 4), ("bed1", 4),
                      ("bd2", 4), ("gd2", 4), ("bed2", 4),
                      ("bd3", 6), ("bm", 6), ("z0", 1), ("lneps", 1)]:
            self._reg_vec(nm, n)
        self.d_vecs = nc.dram_tensor("vecs", [128, self._vec_cols], F32,
                                     kind="ExternalInput")
        self.d_out = nc.dram_tensor("tok_loss", [1, NT], F32,
                                    kind="ExternalOutput")

    def vcol(self, name, t=0):
        s, n = self.vec_map[name]
        assert t < n
        return self.vecs_sb[:, s + t:s + t + 1]

    # ---- helpers --------------------------------------------------------
    def load_weight_tiles(self, pool, wname):
        nc = self.nc
        off, ntiles, M = self.blob_off[wname]
        tiles = []
        for k in range(ntiles):
            t = pool.tile([128, M], BF16, tag=f"w_{wname}_{k}",
                          name=f"w_{wname}_{k}")
            nc.sync.dma_start(
                t[:], self.d_blob[:, off + k * M:off + (k + 1) * M])
            tiles.append(t)
        return tiles

    def layer_bf(self, chs_in, w_tiles, M_out, *, bias, ln=None, relu=False,
                 out_slot=0, out_override=None, out_dtype=BF16):
        """Breadth-first layer over len(chs_in) chunks.

        chs_in: {c: [in_aps]} (bf16 [128, CH]).  Returns {c: [out_aps]}.
        out_override: {c: ap} for single-tile output written to a big tensor.
        """
        nc, CH = self.nc, self.CH
        cs_list = sorted(chs_in.keys())
        n_k = len(chs_in[cs_list[0]])
        n_m = M_out // 128
        tp = self.tmp_pool
        outs = {}
        for c in cs_list:
            if out_override is not None:
                outs[c] = [out_override[c]]
            else:
                outs[c] = [tp.tile([128, CH], out_dtype,
                                   tag=f"t{out_slot}m{m}c{c}",
                                   name=f"t{out_slot}m{m}c{c}")[:]
                           for m in range(n_m)]
        ys = {c: [] for c in cs_list}
        sts = {}
        if ln is not None:
            for c in cs_list:
                sts[c] = self.sp.tile([33, CH], F32, tag=f"st{c}",
                                      name=f"st{c}")
        # main matmuls + evacuation, m-outer / c-inner
        for m in range(n_m):
            for c in cs_list:
                ps = self.pp.tile([128, CH], F32, tag=f"ps{c}",
                                  name=f"ps{c}")
                for k in range(n_k):
                    nc.tensor.matmul(ps[:],
                                     w_tiles[k][:, m * 128:(m + 1) * 128],
                                     chs_in[c][k], start=(k == 0),
                                     stop=(k == n_k - 1))
                if ln is None:
                    nc.scalar.activation(outs[c][m], ps[:],
                                         AF.Relu if relu else AF.Identity,
                                         bias=self.vcol(bias, m))
                else:
                    y = tp.tile([128, CH], BF16, tag=f"y{m}c{c}",
                                name=f"y{m}c{c}")
                    nc.scalar.activation(y[:], ps[:], AF.Identity,
                                         bias=self.vcol(bias, m))
                    sq = tp.tile([128, CH], BF16, tag=f"ub{c}",
                                 name=f"sq{c}")
                    nc.scalar.activation(sq[:], ps[:], AF.Square,
                                         bias=self.vcol(bias, m))
                    ys[c].append(y)
                    # stats: row0 += sum(y), row1 += sum(sq)
                    st = sts[c]
                    nc.tensor.matmul(st[:], self.onesA[:, 0:33], y[:],
                                     start=(m == 0), stop=False,
                                     skip_group_check=(m != 0))
                    nc.tensor.matmul(st[:], self.onesB[:, 0:33], sq[:],
                                     start=False, stop=(m == n_m - 1),
                                     skip_group_check=(m != n_m - 1))
        if ln is None:
            return outs
        g_nm, be_nm = ln
        inv_f = 1.0 / M_out
        # row math per chunk: all compute rows live on partition 0,
        # packed along the free dim (seg0 = mean->mr, seg1 = var->rstd);
        # the Sigma(y^2) psum row is read at partition 32 (quadrant start).
        rbs = {}
        for c in cs_list:
            st = sts[c]
            r = self.row_pool.tile([1, 2 * CH], F32, tag=f"rows{c}",
                                   name=f"rows{c}")
            s0 = r[0:1, 0:CH]
            s1 = r[0:1, CH:2 * CH]
            nc.vector.tensor_scalar_mul(s0, st[0:1, :], inv_f)
            nc.vector.scalar_tensor_tensor(s1, s0, -1.0, s0,
                                           ALU.mult, ALU.mult)
            nc.vector.scalar_tensor_tensor(s1, st[32:33, :], inv_f, s1,
                                           ALU.mult, ALU.add)
            nc.scalar.activation(s1, s1, AF.Sqrt,
                                 bias=self.vcol("lneps")[0:1])
            nc.vector.reciprocal(s1, s1)
            nc.vector.tensor_mul(s0, s0, s1)
            rrA = self.row_pool.tile([1, CH], BF16, tag=f"rrA{c}",
                                     name=f"rrA{c}")
            rrB = self.row_pool.tile([1, CH], BF16, tag=f"rrB{c}",
                                     name=f"rrB{c}")
            nc.vector.tensor_copy(rrA[:], s1)
            nc.vector.tensor_copy(rrB[:], s0)
            rb = self.tmp_pool.tile([128, CH], BF16, tag=f"rb{c}",
                                    name=f"rb{c}")
            mrb = self.tmp_pool.tile([128, CH], BF16, tag=f"mrb{c}",
                                     name=f"mrb{c}")
            nc.gpsimd.partition_broadcast(rb[:], rrA[:])
            nc.gpsimd.partition_broadcast(mrb[:], rrB[:])
            rbs[c] = (rb, mrb)
        # apply
        for m in range(n_m):
            for c in cs_list:
                rb, mrb = rbs[c]
                u = self.tmp_pool.tile([128, CH], BF16, tag=f"ub{c}",
                                       name=f"ub{c}")
                nc.vector.tensor_mul(u[:], ys[c][m][:], rb[:])
                nc.vector.tensor_sub(u[:], u[:], mrb[:])
                nc.scalar.activation(outs[c][m], u[:],
                                     AF.Relu if relu else AF.Identity,
                                     bias=self.vcol(be_nm, m),
                                     scale=self.vcol(g_nm, m))
        return outs

    def encode_bf(self, chs_in, out_override, out_dtype):
        h = self.layer_bf(chs_in, self.w_sb["W1"], H, bias="b1",
                          ln=("g1", "be1"), relu=True, out_slot=0)
        a = self.layer_bf(h, self.w_sb["Wvo"], H, bias="bvo", out_slot=1)
        g = self.layer_bf(a, self.w_sb["W2"], H, bias="b2",
                          ln=("g2", "be2"), relu=True, out_slot=0)
        zl = self.layer_bf(g, self.w_sb["W3"], L, bias="b3",
                           ln=("g3", "be3"), out_slot=1)
        e = self.layer_bf(zl, self.w_sb["W4"], H, bias="b4",
                          ln=("g4", "be4"), relu=True, out_slot=0)
        self.layer_bf(e, self.w_sb["W5"], L, bias="b5", ln=("g5", "be5"),
                      out_override=out_override, out_dtype=out_dtype)

    # ---- main build -----------------------------------------------------
    def build(self):
        nc = self.nc
        NT, CH, S, BL = self.NTOK, self.CH, self.S, self.BL
        NCH = self.NCH
        self.declare()
        import os as _os
        skip_rec = _os.environ.get("SKIP_REC") == "1"
        with tile.TileContext(nc) as tc:
            with (
                tc.tile_pool(name="const", bufs=1) as const_pool,
                tc.tile_pool(name="wenc", bufs=1) as wenc_pool,
                tc.tile_pool(name="big", bufs=1) as big_pool,
                tc.tile_pool(name="tmp", bufs=1) as tmp_pool,
                tc.tile_pool(name="rows", bufs=1) as row_pool,
            ):
                self.tmp_pool, self.row_pool = tmp_pool, row_pool

                # constants
                self.onesA = const_pool.tile([128, 33], BF16, name="onesA")
                self.onesB = const_pool.tile([128, 33], BF16, name="onesB")
                nc.vector.memset(self.onesA[:], 0.0)
                nc.vector.memset(self.onesA[:, 0:1], 1.0)
                nc.vector.memset(self.onesB[:], 0.0)
                nc.vector.memset(self.onesB[:, 32:33], 1.0)
                self.vecs_sb = const_pool.tile([128, self._vec_cols], F32)
                nc.sync.dma_start(self.vecs_sb[:], self.d_vecs[:, :])
                self.z016 = const_pool.tile([128, 1], BF16, name="z016")
                nc.vector.tensor_copy(self.z016[:], self.vcol("z0"))

                self.w_sb = {}
                for k in ("W1", "Wvo", "W2", "W3", "W4", "W5"):
                    self.w_sb[k] = self.load_weight_tiles(wenc_pool, k)

                latx = big_pool.tile([128, NT], BF16, tag="latx", name="latx")
                latm = big_pool.tile([128, NT], BF16, tag="latm", name="latm")
                lrow = {c: row_pool.tile([1, 2 * CH], F32, tag=f"lrow{c}",
                                         name=f"lrow{c}")
                        for c in range(NCH)}

                xt_cm = tc.tile_pool(name="xtp", bufs=1)
                xt_pool = xt_cm.__enter__()
                xt = [xt_pool.tile([128, NT], BF16, tag=f"xt{k}",
                                   name=f"xt{k}") for k in range(6)]
                xt_off = self.blob_off["xt"][0]
                for k in range(6):
                    nc.sync.dma_start(
                        xt[k][:],
                        self.d_blob[:, xt_off + k * NT:xt_off + (k + 1) * NT])
                pt_cm = tc.tile_pool(name="ptp", bufs=1)
                pt_pool = pt_cm.__enter__()
                ptw = pt_pool.tile([128, 6 * NT], BF16, tag="ptw",
                                   name="ptw")
                pt = [ptw[:, m * NT:(m + 1) * NT] for m in range(6)]

                # ==== phase 0: pt = Wm^T x + bm  (breadth-first)
                with (
                    tc.tile_pool(name="ps0", bufs=1, space="PSUM") as pp0,
                    tc.tile_pool(name="wm", bufs=1) as wm_pool,
                ):
                    wm = self.load_weight_tiles(wm_pool, "Wm")
                    for m in range(6):
                        for c in range(NCH):
                            cs = slice(c * CH, (c + 1) * CH)
                            ps = pp0.tile([128, CH], F32, tag=f"p{c}",
                                          name=f"p{c}")
                            for k in range(6):
                                nc.tensor.matmul(
                                    ps[:], wm[k][:, m * 128:(m + 1) * 128],
                                    xt[k][:, cs], start=(k == 0),
                                    stop=(k == 5))
                            nc.scalar.activation(pt[m][:, cs], ps[:],
                                                 AF.Identity,
                                                 bias=self.vcol("bm", m))

                # ==== phase 1: recurrence (split PSUM groups)
                memw = big_pool.tile([128, 6 * NT], BF16, tag="memw",
                                     name="memw")
                memsb = [memw[:, k * NT:(k + 1) * NT] for k in range(6)]
                ptv = ptw[:].rearrange("p (m b s) -> p m b s", m=6, b=BL)
                memv = memw[:].rearrange("p (m b s) -> p m b s", m=6, b=BL)
                with (
                    tc.tile_pool(name="um", bufs=1) as um_pool,
                    tc.tile_pool(name="recps", bufs=1, space="PSUM") as rps,
                ):
                    um = self.load_weight_tiles(um_pool, "Um")
                    id_off = self.blob_off["id"][0]
                    id_sb = um_pool.tile([128, 128], BF16, name="id_sb")
                    nc.sync.dma_start(id_sb[:],
                                      self.d_blob[:, id_off:id_off + 128])
                    G = 3 * BL
                    if skip_rec:
                        nc.vector.memset(memw[:], 0.1)
                    nc.scalar.activation(memv[:, 0:3, :, 0],
                                         ptv[:, 0:3, :, 0], AF.Tanh)
                    nc.scalar.activation(memv[:, 3:6, :, 0],
                                         ptv[:, 3:6, :, 0], AF.Tanh)
                    nsteps = 2 if skip_rec else S
                    for j in range(1, nsteps):
                        pss = []
                        for g in range(2):
                            ps = rps.tile([128, G], F32, tag=f"rps{g}",
                                          name=f"rps{g}", bufs=2,
                                          padded_shape=[128, 512])
                            nc.tensor.matmul(ps[:], id_sb[:],
                                             ptv[:, 3 * g:3 * g + 3, :, j],
                                             start=True, stop=False)
                            pss.append(ps)
                        for g in range(2):
                            ps = pss[g]
                            for k in range(6):
                                for mi in range(3):
                                    m = 3 * g + mi
                                    last = (k == 5 and mi == 2)
                                    nc.tensor.matmul(
                                        ps[:, mi * BL:(mi + 1) * BL],
                                        um[k][:, m * 128:(m + 1) * 128],
                                        memv[:, k, :, j - 1],
                                        start=False, stop=last,
                                        skip_group_check=not last)
                            psv = ps[:].rearrange("p (m b) -> p m b", m=3)
                            nc.scalar.activation(
                                memv[:, 3 * g:3 * g + 3, :, j], psv[:],
                                AF.Tanh)
                pt_cm.__exit__(None, None, None)

                # ==== MLP phases (breadth-first over chunks)
                mlp_ps = tc.tile_pool(name="mps", bufs=1, space="PSUM")
                self.pp = mlp_ps.__enter__()
                mlp_sp = tc.tile_pool(name="msp", bufs=1, space="PSUM")
                self.sp = mlp_sp.__enter__()

                allc = list(range(NCH))
                xt_chs = {c: [xt[k][:, c * CH:(c + 1) * CH] for k in range(6)]
                          for c in allc}
                lat_ov = {c: latx[:, c * CH:(c + 1) * CH] for c in allc}

                # phase 2: encode(x) -> latx (bf16)
                self.encode_bf(xt_chs, lat_ov, BF16)

                # phase 3: decode + recon + trans
                with tc.tile_pool(name="wdec", bufs=1) as wdec_pool:
                    wd1 = self.load_weight_tiles(wdec_pool, "Wd1")
                    wd2 = self.load_weight_tiles(wdec_pool, "Wd2")
                    wd3 = self.load_weight_tiles(wdec_pool, "Wd3")
                    lat16 = {c: [latx[:, c * CH:(c + 1) * CH]]
                             for c in allc}
                    h1 = self.layer_bf(lat16, wd1, H, bias="bd1",
                                       ln=("gd1", "bed1"), relu=True,
                                       out_slot=0)
                    h2 = self.layer_bf(h1, wd2, H, bias="bd2",
                                       ln=("gd2", "bed2"), relu=True,
                                       out_slot=1)
                    # Wd3 + recon: row0 of lrow[c]
                    rsts = {c: self.sp.tile([33, CH], F32, tag=f"st{c}",
                                            name=f"st{c}") for c in allc}
                    for m in range(6):
                        for c in allc:
                            cs = slice(c * CH, (c + 1) * CH)
                            ps = self.pp.tile([128, CH], F32, tag=f"ps{c}",
                                              name=f"ps{c}")
                            for k in range(4):
                                nc.tensor.matmul(
                                    ps[:], wd3[k][:, m * 128:(m + 1) * 128],
                                    h2[c][k], start=(k == 0), stop=(k == 3))
                            r = tmp_pool.tile([128, CH], BF16,
                                              tag=f"ub{c}", name=f"rr{c}")
                            nc.vector.scalar_tensor_tensor(
                                r[:], ps[:], self.vcol("bd3", m),
                                xt[m][:, cs], ALU.add, ALU.subtract)
                            nc.vector.tensor_mul(r[:], r[:], r[:])
                            st = rsts[c]
                            nc.tensor.matmul(st[:], self.onesA[:, 0:33],
                                             r[:],
                                             start=(m == 0), stop=False,
                                             skip_group_check=(m != 0))
                            if m == 5:
                                # trans into row1: dif of latx vs prev token
                                dif = tmp_pool.tile([128, CH], BF16,
                                                    tag=f"rb{c}",
                                                    name=f"dif{c}")
                                cst = c * CH
                                if cst == 0:
                                    nc.vector.tensor_sub(
                                        dif[:, 1:CH],
                                        latx[:, cst + 1:cst + CH],
                                        latx[:, cst:cst + CH - 1])
                                    nc.vector.tensor_sub(dif[:, 0:1],
                                                         latx[:, 0:1],
                                                         self.z016[:])
                                else:
                                    nc.vector.tensor_sub(
                                        dif[:], latx[:, cst:cst + CH],
                                        latx[:, cst - 1:cst + CH - 1])
                                for b in range(BL):
                                    c0 = b * S
                                    if c0 > 0 and cst <= c0 <= cst + CH - 1:
                                        nc.vector.tensor_sub(
                                            dif[:, c0 - cst:c0 - cst + 1],
                                            latx[:, c0:c0 + 1],
                                            self.z016[:])
                                dif2 = tmp_pool.tile([128, CH], BF16,
                                                     tag=f"ub{c}",
                                                     name=f"dif2{c}")
                                nc.vector.tensor_mul(dif2[:], dif[:], dif[:])
                                nc.tensor.matmul(st[:], self.onesB[:, 0:33],
                                                 dif2[:], start=False,
                                                 stop=True)
                    for c in allc:
                        st = rsts[c]
                        # lrow[c]: recon/D @ seg0, trans/L @ seg1
                        nc.vector.tensor_scalar(lrow[c][0:1, 0:CH],
                                                st[0:1, :],
                                                1.0 / D, 10.0, ALU.mult,
                                                ALU.min)
                        nc.vector.tensor_scalar(lrow[c][0:1, CH:2 * CH],
                                                st[32:33, :],
                                                1.0 / L, 10.0, ALU.mult,
                                                ALU.min)
                xt_cm.__exit__(None, None, None)

                # phase 4: encode(mem) -> latm (bf16)
                mem_chs = {c: [memsb[k][:, c * CH:(c + 1) * CH]
                               for k in range(6)] for c in allc}
                latm_ov = {c: latm[:, c * CH:(c + 1) * CH] for c in allc}
                self.encode_bf(mem_chs, latm_ov, BF16)

                # phase 5: ctx + combine (rows on partition 0, free-dim
                # segments; Sigma rows read from psum at partitions 0/32)
                csts = {c: self.sp.tile([33, CH], F32, tag=f"st{c}",
                                        name=f"st{c}") for c in allc}
                rowsd = {}
                for c in allc:
                    cs = slice(c * CH, (c + 1) * CH)
                    st = csts[c]
                    u = tmp_pool.tile([128, CH], BF16, tag=f"y0c{c}",
                                      name=f"cu{c}")
                    nc.vector.tensor_mul(u[:], latx[:, cs], latx[:, cs])
                    nc.tensor.matmul(st[:], self.onesA[:, 0:33], u[:],
                                     start=True, stop=False)
                    u2 = tmp_pool.tile([128, CH], BF16, tag=f"y1c{c}",
                                       name=f"cu2{c}")
                    nc.vector.tensor_mul(u2[:], latm[:, cs], latm[:, cs])
                    nc.tensor.matmul(st[:], self.onesB[:, 0:33], u2[:],
                                     start=False, stop=True)
                for c in allc:
                    st = csts[c]
                    # rows: seg0 = 1/max(sqrt(nx),eps) * later terms,
                    # seg1 = 1/max(sqrt(nm),eps)
                    r = self.row_pool.tile([1, 2 * CH], F32, tag=f"rows{c}",
                                           name=f"rows{c}")
                    rowsd[c] = r
                    s0 = r[0:1, 0:CH]
                    s1 = r[0:1, CH:2 * CH]
                    nc.scalar.activation(s0, st[0:1, :], AF.Sqrt)
                    nc.scalar.activation(s1, st[32:33, :], AF.Sqrt)
                    nc.vector.tensor_scalar_max(r[0:1, :], r[0:1, :],
                                                NORM_EPS)
                    nc.vector.reciprocal(r[0:1, :], r[0:1, :])
                csts2 = {c: self.sp.tile([33, CH], F32, tag=f"st{c}",
                                         name=f"st{c}") for c in allc}
                for c in allc:
                    cs = slice(c * CH, (c + 1) * CH)
                    u3 = tmp_pool.tile([128, CH], BF16, tag=f"y2c{c}",
                                       name=f"cu3{c}")
                    nc.vector.tensor_mul(u3[:], latx[:, cs], latm[:, cs])
                    nc.tensor.matmul(csts2[c][:], self.onesA[:, 0:33], u3[:],
                                     start=True, stop=True)
                for c in allc:
                    cs = slice(c * CH, (c + 1) * CH)
                    r = rowsd[c]
                    s0 = r[0:1, 0:CH]
                    s1 = r[0:1, CH:2 * CH]
                    # s0 = cos = dot * rx * rm
                    nc.vector.tensor_mul(s0, csts2[c][0:1, :], s0)
                    nc.vector.tensor_mul(s0, s0, s1)
                    # s0 = clip(1 - cos, 0, 10)
                    nc.vector.tensor_scalar(s0, s0, -1.0, 1.0,
                                            ALU.mult, ALU.add)
                    nc.vector.tensor_scalar(s0, s0, 0.0, 10.0,
                                            ALU.max, ALU.min)
                    # s1 = recon + 0.3*trans + 0.3*ctx
                    nc.vector.scalar_tensor_tensor(
                        s1, lrow[c][0:1, CH:2 * CH], 0.3,
                        lrow[c][0:1, 0:CH], ALU.mult, ALU.add)
                    nc.vector.scalar_tensor_tensor(
                        s1, s0, 0.3, s1, ALU.mult, ALU.add)
                    nc.sync.dma_start(self.d_out[:, cs], s1)

                mlp_sp.__exit__(None, None, None)
                mlp_ps.__exit__(None, None, None)
        nc.compile()
        return nc

# ---------------------------------------------------------------- runner

_CACHE = {}


def _get_built(S, BL):
    key = (S, BL)
    if key not in _CACHE:
        kb = _KB(S, BL)
        kb.build()
        _CACHE[key] = kb
    return _CACHE[key]


def _host_inputs(kb, inputs):
    S, BL = kb.S, kb.BL
    w = {k: np.asarray(v, np.float32) for k, v in inputs.items()}
    Wvo = w["Wv"] @ w["Wo"]
    bvo = w["bv"] @ w["Wo"] + w["bo"]
    wd = dict(w)
    wd["Wvo"], wd["bvo"] = Wvo, bvo
    z0 = _encode_np(np.zeros((1, D), np.float32), wd)[0]

    vecs = _pack_cols(w["b1"], w["g1"], w["be1"], bvo,
                      w["b2"], w["g2"], w["be2"],
                      w["b3"], w["g3"], w["be3"],
                      w["b4"], w["g4"], w["be4"],
                      w["b5"], w["g5"], w["be5"],
                      w["bd1"], w["gd1"], w["bed1"],
                      w["bd2"], w["gd2"], w["bed2"],
                      w["bd3"], w["bm"], z0,
                      np.full(128, LN_EPS, np.float32))

    def b16(x):
        return np.ascontiguousarray(x.astype(ml_dtypes.bfloat16))

    wd["id"] = np.eye(128, dtype=np.float32)
    blob_off, nblob = kb.blob_layout()
    wblob = np.zeros((128, nblob), ml_dtypes.bfloat16)
    for name, (off, ntiles, M) in blob_off.items():
        if name == "xt":
            continue
        wsrc = np.asarray(wd[name], np.float32)
        for k in range(ntiles):
            wblob[:, off + k * M:off + (k + 1) * M] = b16(
                wsrc[k * 128:(k + 1) * 128, :])

    seqs = np.asarray(inputs["sequences"], np.float32)
    xt_off, xnt, xm = blob_off["xt"]
    in_maps = []
    for c in range(NCORES):
        xs = seqs[c * BL:(c + 1) * BL, :S, :]
        xt = b16(xs.reshape(BL * S, D).T)           # [D, NTOK]
        blob = wblob.copy()
        for k in range(xnt):
            blob[:, xt_off + k * xm:xt_off + (k + 1) * xm] = \
                xt[k * 128:(k + 1) * 128, :]
        in_maps.append(dict(blob16=blob, vecs=vecs))
    return in_maps


def _l2_term(inputs):
    names = ["W1", "b1", "g1", "be1", "Wv", "bv", "Wo", "bo", "W2", "b2", "g2",
             "be2", "W3", "b3", "g3", "be3", "W4", "b4", "g4", "be4", "W5",
             "b5", "g5", "be5", "Wd1", "bd1", "gd1", "bed1", "Wd2", "bd2",
             "gd2", "bed2", "Wd3", "bd3", "Wm", "Um", "bm"]
    l2 = sum(np.linalg.norm(np.asarray(inputs[n], np.float64)) for n in names)
    return float(np.clip(l2, 0.0, 10.0))


def _combine(kb, res, inputs):
    tok = np.concatenate([res.results[c]["tok_loss"].reshape(-1)
                          for c in range(NCORES)])
    l2 = _l2_term(inputs)
    per_tok = np.clip(tok.astype(np.float64) + 1e-4 * l2, 0.0, 100.0)
    nb = kb.BL * NCORES
    return np.float32(per_tok.sum() / nb)


def kernel(**inputs):
    seqs = np.asarray(inputs["sequences"])
    S = seqs.shape[1]
    BL = seqs.shape[0] // NCORES
    kb = _get_built(S, BL)
    in_maps = _host_inputs(kb, inputs)
    res = run_bass_kernel_spmd(kb.nc, in_maps, list(range(NCORES)))
    return _combine(kb, res, inputs)


# revision 35
# speedup vs baseline: 2.4041x; 1.0120x over previous
"""Trainium2 Bass kernel for nn_EmotionalEmbeddingSpace.

Sharding: data-parallel over batch B=16 across 8 cores (2 sequences/core).
Layout on device: transposed - features on partitions, tokens on the free dim.

Per core (BL=2 sequences, NTOK=BL*S tokens):
  pt   = Wm^T @ x^T + bm                 (bulk matmul)
  mem_j = tanh(pt_j + Um^T @ mem_{j-1})  (serial; split PSUM groups so tanh
                                          hides behind the LDWEIGHTS stream)
  latx = encode(x), latm = encode(mem)   (MLP chains, breadth-first across
                                          4 token chunks so cross-engine
                                          latency hides behind other chunks)
  recon/trans/ctx per-token losses -> tok_loss[NTOK] -> host adds l2 term.

Matmul inputs bf16 (f32 PSUM); LN stats from bf16 y/sq via dual ACT
evacuation (Identity + Square); row math f32.
"""

import sys

sys.path.insert(0, "/opt/trn_rl_repo")

import numpy as np
import ml_dtypes

import concourse.bass as bass
import concourse.bacc as bacc
import concourse.mybir as mybir
import concourse.tile as tile
from concourse.bass_utils import run_bass_kernel_spmd

F32 = mybir.dt.float32
BF16 = mybir.dt.bfloat16
AF = mybir.ActivationFunctionType
ALU = mybir.AluOpType

B, S_FULL, D, H, L = 16, 1024, 768, 512, 128
NCORES = 8
LN_EPS = 1e-5
NORM_EPS = 1e-8


# ---------------------------------------------------------------- host prep

def _pack_cols(*vecs):
    cols = []
    for v in vecs:
        v = np.asarray(v, np.float32).reshape(-1, 128)
        cols.append(v.T)
    return np.ascontiguousarray(np.concatenate(cols, axis=1))


def _ln_np(x, g, b, eps=LN_EPS):
    m = x.mean(-1, keepdims=True)
    v = ((x - m) ** 2).mean(-1, keepdims=True)
    return (x - m) / np.sqrt(v + eps) * g + b


def _encode_np(t, w):
    h = np.maximum(_ln_np(t @ w["W1"] + w["b1"], w["g1"], w["be1"]), 0)
    a = h @ w["Wvo"] + w["bvo"]
    g = np.maximum(_ln_np(a @ w["W2"] + w["b2"], w["g2"], w["be2"]), 0)
    zl = _ln_np(g @ w["W3"] + w["b3"], w["g3"], w["be3"])
    e = np.maximum(_ln_np(zl @ w["W4"] + w["b4"], w["g4"], w["be4"]), 0)
    return _ln_np(e @ w["W5"] + w["b5"], w["g5"], w["be5"])


# ---------------------------------------------------------------- builder

class _KB:
    def __init__(self, S=S_FULL, BL=B // NCORES):
        self.S, self.BL = S, BL
        self.NTOK = S * BL
        self.CH = min(512, self.NTOK)
        self.NCH = self.NTOK // self.CH
        self.nc = bacc.Bacc("TRN2", target_bir_lowering=False, debug=False,
                            num_devices=NCORES)
        self.vec_map = {}
        self._vec_cols = 0

    def _reg_vec(self, name, ntiles):
        self.vec_map[name] = (self._vec_cols, ntiles)
        self._vec_cols += ntiles

    WSHAPES = dict(W1=(D, H), Wvo=(H, H), W2=(H, H), W3=(H, L), W4=(L, H),
                   W5=(H, L), Wd1=(L, H), Wd2=(H, H), Wd3=(H, D),
                   Wm=(D, D), Um=(D, D))

    def blob_layout(self):
        """Column layout of the single bf16 input blob: xt tiles first,
        then each weight as K//128 row-tiles of M columns, then identity."""
        NT = self.NTOK
        entries = [("xt", 6, NT)]
        for k, (K, M) in self.WSHAPES.items():
            entries.append((k, K // 128, M))
        entries.append(("id", 1, 128))
        entries.append(("vecs", 1, self._vec_cols))
        off = {}
        pos = 0
        for name, ntiles, M in entries:
            off[name] = (pos, ntiles, M)
            pos += ntiles * M
        return off, pos

    def declare(self):
        nc = self.nc
        NT = self.NTOK
        for nm, n in [("b1", 4), ("g1", 4), ("be1", 4), ("bvo", 4),
                      ("b2", 4), ("g2", 4), ("be2", 4),
                      ("b3", 1), ("g3", 1), ("be3", 1),
                      ("b4", 4), ("g4", 4), ("be4", 4),
                      ("b5", 1), ("g5", 1), ("be5", 1),
                      ("bd1", 4), ("gd1", 4), ("bed1", 4),
                      ("bd2", 4), ("gd2", 4), ("bed2", 4),
                      ("bd3", 6), ("bm", 6), ("z0", 1), ("lneps", 1)]:
            self._reg_vec(nm, n)
        self.blob_off, nblob = self.blob_layout()
        self.d_blob = nc.dram_tensor("blob16", [128, nblob], BF16,
                                     kind="ExternalInput")
        self.d_out = nc.dram_tensor("tok_loss", [1, NT], F32,
                                    kind="ExternalOutput")

    def vcol(self, name, t=0):
        s, n = self.vec_map[name]
        assert t < n
        return self.vecs_sb[:, s + t:s + t + 1]

    # ---- helpers --------------------------------------------------------
    def load_weight_tiles(self, pool, wname):
        nc = self.nc
        off, ntiles, M = self.blob_off[wname]
        tiles = []
        for k in range(ntiles):
            t = pool.tile([128, M], BF16, tag=f"w_{wname}_{k}",
                          name=f"w_{wname}_{k}")
            nc.sync.dma_start(
                t[:], self.d_blob[:, off + k * M:off + (k + 1) * M])
            tiles.append(t)
        return tiles

    def layer_bf(self, chs_in, w_tiles, M_out, *, bias, ln=None, relu=False,
                 out_slot=0, out_override=None, out_dtype=BF16):
        """Breadth-first layer over len(chs_in) chunks.

        chs_in: {c: [in_aps]} (bf16 [128, CH]).  Returns {c: [out_aps]}.
        out_override: {c: ap} for single-tile output written to a big tensor.
        """
        nc, CH = self.nc, self.CH
        cs_list = sorted(chs_in.keys())
        n_k = len(chs_in[cs_list[0]])
        n_m = M_out // 128
        tp = self.tmp_pool
        outs = {}
        for c in cs_list:
            if out_override is not None:
                outs[c] = [out_override[c]]
            else:
                outs[c] = [tp.tile([128, CH], out_dtype,
                                   tag=f"t{out_slot}m{m}c{c}",
                                   name=f"t{out_slot}m{m}c{c}")[:]
                           for m in range(n_m)]
        ys = {c: [] for c in cs_list}
        sts = {}
        if ln is not None:
            for c in cs_list:
                sts[c] = self.sp.tile([33, CH], F32, tag=f"st{c}",
                                      name=f"st{c}")
        # main matmuls + evacuation, m-outer / c-inner
        for m in range(n_m):
            for c in cs_list:
                ps = self.pp.tile([128, CH], F32, tag=f"ps{c}",
                                  name=f"ps{c}")
                for k in range(n_k):
                    nc.tensor.matmul(ps[:],
                                     w_tiles[k][:, m * 128:(m + 1) * 128],
                                     chs_in[c][k], start=(k == 0),
                                     stop=(k == n_k - 1))
                if ln is None:
                    nc.scalar.activation(outs[c][m], ps[:],
                                         AF.Relu if relu else AF.Identity,
                                         bias=self.vcol(bias, m))
                else:
                    y = tp.tile([128, CH], BF16, tag=f"y{m}c{c}",
                                name=f"y{m}c{c}")
                    nc.scalar.activation(y[:], ps[:], AF.Identity,
                                         bias=self.vcol(bias, m))
                    sq = tp.tile([128, CH], BF16, tag=f"ub{c}",
                                 name=f"sq{c}")
                    nc.scalar.activation(sq[:], ps[:], AF.Square,
                                         bias=self.vcol(bias, m))
                    ys[c].append(y)
                    # stats: row0 += sum(y), row1 += sum(sq)
                    st = sts[c]
                    nc.tensor.matmul(st[:], self.onesA[:, 0:33], y[:],
                                     start=(m == 0), stop=False,
                                     skip_group_check=(m != 0))
                    nc.tensor.matmul(st[:], self.onesB[:, 0:33], sq[:],
                                     start=False, stop=(m == n_m - 1),
                                     skip_group_check=(m != n_m - 1))
        if ln is None:
            return outs
        g_nm, be_nm = ln
        inv_f = 1.0 / M_out
        # row math per chunk: all compute rows live on partition 0,
        # packed along the free dim (seg0 = mean->mr, seg1 = var->rstd);
        # the Sigma(y^2) psum row is read at partition 32 (quadrant start).
        rbs = {}
        for c in cs_list:
            st = sts[c]
            r = self.row_pool.tile([1, 2 * CH], F32, tag=f"rows{c}",
                                   name=f"rows{c}")
            s0 = r[0:1, 0:CH]
            s1 = r[0:1, CH:2 * CH]
            nc.vector.tensor_scalar_mul(s0, st[0:1, :], inv_f)
            nc.vector.scalar_tensor_tensor(s1, s0, -1.0, s0,
                                           ALU.mult, ALU.mult)
            nc.vector.scalar_tensor_tensor(s1, st[32:33, :], inv_f, s1,
                                           ALU.mult, ALU.add)
            nc.scalar.activation(s1, s1, AF.Sqrt,
                                 bias=self.vcol("lneps")[0:1])
            nc.vector.reciprocal(s1, s1)
            nc.vector.tensor_mul(s0, s0, s1)
            rrA = self.row_pool.tile([1, CH], BF16, tag=f"rrA{c}",
                                     name=f"rrA{c}")
            rrB = self.row_pool.tile([1, CH], BF16, tag=f"rrB{c}",
                                     name=f"rrB{c}")
            nc.vector.tensor_copy(rrA[:], s1)
            nc.vector.tensor_copy(rrB[:], s0)
            rb = self.tmp_pool.tile([128, CH], BF16, tag=f"rb{c}",
                                    name=f"rb{c}")
            mrb = self.tmp_pool.tile([128, CH], BF16, tag=f"mrb{c}",
                                     name=f"mrb{c}")
            nc.gpsimd.partition_broadcast(rb[:], rrA[:])
            nc.gpsimd.partition_broadcast(mrb[:], rrB[:])
            rbs[c] = (rb, mrb)
        # apply
        for m in range(n_m):
            for c in cs_list:
                rb, mrb = rbs[c]
                u = self.tmp_pool.tile([128, CH], BF16, tag=f"ub{c}",
                                       name=f"ub{c}")
                nc.vector.tensor_mul(u[:], ys[c][m][:], rb[:])
                nc.vector.tensor_sub(u[:], u[:], mrb[:])
                nc.scalar.activation(outs[c][m], u[:],
                                     AF.Relu if relu else AF.Identity,
                                     bias=self.vcol(be_nm, m),
                                     scale=self.vcol(g_nm, m))
        return outs

    def encode_bf(self, chs_in, out_override, out_dtype):
        h = self.layer_bf(chs_in, self.w_sb["W1"], H, bias="b1",
                          ln=("g1", "be1"), relu=True, out_slot=0)
        a = self.layer_bf(h, self.w_sb["Wvo"], H, bias="bvo", out_slot=1)
        g = self.layer_bf(a, self.w_sb["W2"], H, bias="b2",
                          ln=("g2", "be2"), relu=True, out_slot=0)
        zl = self.layer_bf(g, self.w_sb["W3"], L, bias="b3",
                           ln=("g3", "be3"), out_slot=1)
        e = self.layer_bf(zl, self.w_sb["W4"], H, bias="b4",
                          ln=("g4", "be4"), relu=True, out_slot=0)
        self.layer_bf(e, self.w_sb["W5"], L, bias="b5", ln=("g5", "be5"),
                      out_override=out_override, out_dtype=out_dtype)

    # ---- main build -----------------------------------------------------
    def build(self):
        nc = self.nc
        NT, CH, S, BL = self.NTOK, self.CH, self.S, self.BL
        NCH = self.NCH
        self.declare()
        import os as _os
        skip_rec = _os.environ.get("SKIP_REC") == "1"
        with tile.TileContext(nc) as tc:
            with (
                tc.tile_pool(name="const", bufs=1) as const_pool,
                tc.tile_pool(name="wenc", bufs=1) as wenc_pool,
                tc.tile_pool(name="big", bufs=1) as big_pool,
                tc.tile_pool(name="tmp", bufs=1) as tmp_pool,
                tc.tile_pool(name="rows", bufs=1) as row_pool,
            ):
                self.tmp_pool, self.row_pool = tmp_pool, row_pool

                # constants
                self.onesA = const_pool.tile([128, 33], BF16, name="onesA")
                self.onesB = const_pool.tile([128, 33], BF16, name="onesB")
                nc.vector.memset(self.onesA[:], 0.0)
                nc.vector.memset(self.onesA[:, 0:1], 1.0)
                nc.vector.memset(self.onesB[:], 0.0)
                nc.vector.memset(self.onesB[:, 32:33], 1.0)
                voff = self.blob_off["vecs"][0]
                vecs16 = const_pool.tile([128, self._vec_cols], BF16,
                                         name="vecs16")
                nc.sync.dma_start(
                    vecs16[:],
                    self.d_blob[:, voff:voff + self._vec_cols])
                self.vecs_sb = const_pool.tile([128, self._vec_cols], F32)
                nc.vector.tensor_copy(self.vecs_sb[:], vecs16[:])
                self.z016 = const_pool.tile([128, 1], BF16, name="z016")
                nc.vector.tensor_copy(self.z016[:], self.vcol("z0"))

                self.w_sb = {}
                for k in ("W1", "Wvo", "W2", "W3", "W4", "W5"):
                    self.w_sb[k] = self.load_weight_tiles(wenc_pool, k)

                latx = big_pool.tile([128, NT], BF16, tag="latx", name="latx")
                latm = big_pool.tile([128, NT], BF16, tag="latm", name="latm")
                lrow = {c: row_pool.tile([1, 2 * CH], F32, tag=f"lrow{c}",
                                         name=f"lrow{c}")
                        for c in range(NCH)}

                xt_cm = tc.tile_pool(name="xtp", bufs=1)
                xt_pool = xt_cm.__enter__()
                xt = [xt_pool.tile([128, NT], BF16, tag=f"xt{k}",
                                   name=f"xt{k}") for k in range(6)]
                xt_off = self.blob_off["xt"][0]
                for k in range(6):
                    nc.sync.dma_start(
                        xt[k][:],
                        self.d_blob[:, xt_off + k * NT:xt_off + (k + 1) * NT])
                pt_cm = tc.tile_pool(name="ptp", bufs=1)
                pt_pool = pt_cm.__enter__()
                ptw = pt_pool.tile([128, 6 * NT], BF16, tag="ptw",
                                   name="ptw")
                pt = [ptw[:, m * NT:(m + 1) * NT] for m in range(6)]

                # ==== phase 0: pt = Wm^T x + bm  (breadth-first)
                with (
                    tc.tile_pool(name="ps0", bufs=1, space="PSUM") as pp0,
                    tc.tile_pool(name="wm", bufs=1) as wm_pool,
                ):
                    wm = self.load_weight_tiles(wm_pool, "Wm")
                    for m in range(6):
                        for c in range(NCH):
                            cs = slice(c * CH, (c + 1) * CH)
                            ps = pp0.tile([128, CH], F32, tag=f"p{c}",
                                          name=f"p{c}")
                            for k in range(6):
                                nc.tensor.matmul(
                                    ps[:], wm[k][:, m * 128:(m + 1) * 128],
                                    xt[k][:, cs], start=(k == 0),
                                    stop=(k == 5))
                            nc.scalar.activation(pt[m][:, cs], ps[:],
                                                 AF.Identity,
                                                 bias=self.vcol("bm", m))

                # ==== phase 1: recurrence (split PSUM groups)
                memw = big_pool.tile([128, 6 * NT], BF16, tag="memw",
                                     name="memw")
                memsb = [memw[:, k * NT:(k + 1) * NT] for k in range(6)]
                ptv = ptw[:].rearrange("p (m b s) -> p m b s", m=6, b=BL)
                memv = memw[:].rearrange("p (m b s) -> p m b s", m=6, b=BL)
                with (
                    tc.tile_pool(name="um", bufs=1) as um_pool,
                    tc.tile_pool(name="recps", bufs=1, space="PSUM") as rps,
                ):
                    um = self.load_weight_tiles(um_pool, "Um")
                    id_off = self.blob_off["id"][0]
                    id_sb = um_pool.tile([128, 128], BF16, name="id_sb")
                    nc.sync.dma_start(id_sb[:],
                                      self.d_blob[:, id_off:id_off + 128])
                    G = 3 * BL
                    if skip_rec:
                        nc.vector.memset(memw[:], 0.1)
                    nc.scalar.activation(memv[:, 0:3, :, 0],
                                         ptv[:, 0:3, :, 0], AF.Tanh)
                    nc.scalar.activation(memv[:, 3:6, :, 0],
                                         ptv[:, 3:6, :, 0], AF.Tanh)
                    nsteps = 2 if skip_rec else S
                    for j in range(1, nsteps):
                        pss = []
                        for g in range(2):
                            ps = rps.tile([128, G], F32, tag=f"rps{g}",
                                          name=f"rps{g}", bufs=2,
                                          padded_shape=[128, 512])
                            nc.tensor.matmul(ps[:], id_sb[:],
                                             ptv[:, 3 * g:3 * g + 3, :, j],
                                             start=True, stop=False)
                            pss.append(ps)
                        for g in range(2):
                            ps = pss[g]
                            for k in range(6):
                                for mi in range(3):
                                    m = 3 * g + mi
                                    last = (k == 5 and mi == 2)
                                    nc.tensor.matmul(
                                        ps[:, mi * BL:(mi + 1) * BL],
                                        um[k][:, m * 128:(m + 1) * 128],
                                        memv[:, k, :, j - 1],
                                        start=False, stop=last,
                                        skip_group_check=not last)
                            psv = ps[:].rearrange("p (m b) -> p m b", m=3)
                            nc.scalar.activation(
                                memv[:, 3 * g:3 * g + 3, :, j], psv[:],
                                AF.Tanh)
                pt_cm.__exit__(None, None, None)

                # ==== MLP phases (breadth-first over chunks)
                mlp_ps = tc.tile_pool(name="mps", bufs=1, space="PSUM")
                self.pp = mlp_ps.__enter__()
                mlp_sp = tc.tile_pool(name="msp", bufs=1, space="PSUM")
                self.sp = mlp_sp.__enter__()

                allc = list(range(NCH))
                xt_chs = {c: [xt[k][:, c * CH:(c + 1) * CH] for k in range(6)]
                          for c in allc}
                lat_ov = {c: latx[:, c * CH:(c + 1) * CH] for c in allc}

                # phase 2: encode(x) -> latx (bf16)
                self.encode_bf(xt_chs, lat_ov, BF16)

                # phase 3: decode + recon + trans
                with tc.tile_pool(name="wdec", bufs=1) as wdec_pool:
                    wd1 = self.load_weight_tiles(wdec_pool, "Wd1")
                    wd2 = self.load_weight_tiles(wdec_pool, "Wd2")
                    wd3 = self.load_weight_tiles(wdec_pool, "Wd3")
                    lat16 = {c: [latx[:, c * CH:(c + 1) * CH]]
                             for c in allc}
                    h1 = self.layer_bf(lat16, wd1, H, bias="bd1",
                                       ln=("gd1", "bed1"), relu=True,
                                       out_slot=0)
                    h2 = self.layer_bf(h1, wd2, H, bias="bd2",
                                       ln=("gd2", "bed2"), relu=True,
                                       out_slot=1)
                    # Wd3 + recon: row0 of lrow[c]
                    rsts = {c: self.sp.tile([33, CH], F32, tag=f"st{c}",
                                            name=f"st{c}") for c in allc}
                    for m in range(6):
                        for c in allc:
                            cs = slice(c * CH, (c + 1) * CH)
                            ps = self.pp.tile([128, CH], F32, tag=f"ps{c}",
                                              name=f"ps{c}")
                            for k in range(4):
                                nc.tensor.matmul(
                                    ps[:], wd3[k][:, m * 128:(m + 1) * 128],
                                    h2[c][k], start=(k == 0), stop=(k == 3))
                            r = tmp_pool.tile([128, CH], BF16,
                                              tag=f"ub{c}", name=f"rr{c}")
                            nc.vector.scalar_tensor_tensor(
                                r[:], ps[:], self.vcol("bd3", m),
                                xt[m][:, cs], ALU.add, ALU.subtract)
                            nc.vector.tensor_mul(r[:], r[:], r[:])
                            st = rsts[c]
                            nc.tensor.matmul(st[:], self.onesA[:, 0:33],
                                             r[:],
                                             start=(m == 0), stop=False,
                                             skip_group_check=(m != 0))
                            if m == 5:
                                # trans into row1: dif of latx vs prev token
                                dif = tmp_pool.tile([128, CH], BF16,
                                                    tag=f"rb{c}",
                                                    name=f"dif{c}")
                                cst = c * CH
                                if cst == 0:
                                    nc.vector.tensor_sub(
                                        dif[:, 1:CH],
                                        latx[:, cst + 1:cst + CH],
                                        latx[:, cst:cst + CH - 1])
                                    nc.vector.tensor_sub(dif[:, 0:1],
                                                         latx[:, 0:1],
                                                         self.z016[:])
                                else:
                                    nc.vector.tensor_sub(
                                        dif[:], latx[:, cst:cst + CH],
                                        latx[:, cst - 1:cst + CH - 1])
                                for b in range(BL):
                                    c0 = b * S
                                    if c0 > 0 and cst <= c0 <= cst + CH - 1:
                                        nc.vector.tensor_sub(
                                            dif[:, c0 - cst:c0 - cst + 1],
                                            latx[:, c0:c0 + 1],
                                            self.z016[:])
                                dif2 = tmp_pool.tile([128, CH], BF16,
                                                     tag=f"ub{c}",
                                                     name=f"dif2{c}")
                                nc.vector.tensor_mul(dif2[:], dif[:], dif[:])
                                nc.tensor.matmul(st[:], self.onesB[:, 0:33],
                                                 dif2[:], start=False,
                                                 stop=True)
                    for c in allc:
                        st = rsts[c]
                        # lrow[c]: recon/D @ seg0, trans/L @ seg1
                        nc.vector.tensor_scalar(lrow[c][0:1, 0:CH],
                                                st[0:1, :],
                                                1.0 / D, 10.0, ALU.mult,
                                                ALU.min)
                        nc.vector.tensor_scalar(lrow[c][0:1, CH:2 * CH],
                                                st[32:33, :],
                                                1.0 / L, 10.0, ALU.mult,
                                                ALU.min)
                xt_cm.__exit__(None, None, None)

                # phase 4: encode(mem) -> latm (bf16)
                mem_chs = {c: [memsb[k][:, c * CH:(c + 1) * CH]
                               for k in range(6)] for c in allc}
                latm_ov = {c: latm[:, c * CH:(c + 1) * CH] for c in allc}
                self.encode_bf(mem_chs, latm_ov, BF16)

                # phase 5: ctx + combine (rows on partition 0, free-dim
                # segments; Sigma rows read from psum at partitions 0/32)
                csts = {c: self.sp.tile([33, CH], F32, tag=f"st{c}",
                                        name=f"st{c}") for c in allc}
                rowsd = {}
                for c in allc:
                    cs = slice(c * CH, (c + 1) * CH)
                    st = csts[c]
                    u = tmp_pool.tile([128, CH], BF16, tag=f"y0c{c}",
                                      name=f"cu{c}")
                    nc.vector.tensor_mul(u[:], latx[:, cs], latx[:, cs])
                    nc.tensor.matmul(st[:], self.onesA[:, 0:33], u[:],
                                     start=True, stop=False)
                    u2 = tmp_pool.tile([128, CH], BF16, tag=f"y1c{c}",
                                       name=f"cu2{c}")
                    nc.vector.tensor_mul(u2[:], latm[:, cs], latm[:, cs])
                    nc.tensor.matmul(st[:], self.onesB[:, 0:33], u2[:],
                                     start=False, stop=True)
                for c in allc:
                    st = csts[c]
                    # rows: seg0 = 1/max(sqrt(nx),eps) * later terms,
                    # seg1 = 1/max(sqrt(nm),eps)
                    r = self.row_pool.tile([1, 2 * CH], F32, tag=f"rows{c}",
                                           name=f"rows{c}")
                    rowsd[c] = r
                    s0 = r[0:1, 0:CH]
                    s1 = r[0:1, CH:2 * CH]
                    nc.scalar.activation(s0, st[0:1, :], AF.Sqrt)
                    nc.scalar.activation(s1, st[32:33, :], AF.Sqrt)
                    nc.vector.tensor_scalar_max(r[0:1, :], r[0:1, :],
                                                NORM_EPS)
                    nc.vector.reciprocal(r[0:1, :], r[0:1, :])
                csts2 = {c: self.sp.tile([33, CH], F32, tag=f"st{c}",
                                         name=f"st{c}") for c in allc}
                for c in allc:
                    cs = slice(c * CH, (c + 1) * CH)
                    u3 = tmp_pool.tile([128, CH], BF16, tag=f"y2c{c}",
                                       name=f"cu3{c}")
                    nc.vector.tensor_mul(u3[:], latx[:, cs], latm[:, cs])
                    nc.tensor.matmul(csts2[c][:], self.onesA[:, 0:33], u3[:],
                                     start=True, stop=True)
                for c in allc:
                    cs = slice(c * CH, (c + 1) * CH)
                    r = rowsd[c]
                    s0 = r[0:1, 0:CH]
                    s1 = r[0:1, CH:2 * CH]
                    # s0 = cos = dot * rx * rm
                    nc.vector.tensor_mul(s0, csts2[c][0:1, :], s0)
                    nc.vector.tensor_mul(s0, s0, s1)
                    # s0 = clip(1 - cos, 0, 10)
                    nc.vector.tensor_scalar(s0, s0, -1.0, 1.0,
                                            ALU.mult, ALU.add)
                    nc.vector.tensor_scalar(s0, s0, 0.0, 10.0,
                                            ALU.max, ALU.min)
                    # s1 = recon + 0.3*trans + 0.3*ctx
                    nc.vector.scalar_tensor_tensor(
                        s1, lrow[c][0:1, CH:2 * CH], 0.3,
                        lrow[c][0:1, 0:CH], ALU.mult, ALU.add)
                    nc.vector.scalar_tensor_tensor(
                        s1, s0, 0.3, s1, ALU.mult, ALU.add)
                    nc.sync.dma_start(self.d_out[:, cs], s1)

                mlp_sp.__exit__(None, None, None)
                mlp_ps.__exit__(None, None, None)
        nc.compile()
        return nc

# ---------------------------------------------------------------- runner

_CACHE = {}


def _get_built(S, BL):
    key = (S, BL)
    if key not in _CACHE:
        kb = _KB(S, BL)
        kb.build()
        _CACHE[key] = kb
    return _CACHE[key]


def _host_inputs(kb, inputs):
    S, BL = kb.S, kb.BL
    w = {k: np.asarray(v, np.float32) for k, v in inputs.items()}
    Wvo = w["Wv"] @ w["Wo"]
    bvo = w["bv"] @ w["Wo"] + w["bo"]
    wd = dict(w)
    wd["Wvo"], wd["bvo"] = Wvo, bvo
    z0 = _encode_np(np.zeros((1, D), np.float32), wd)[0]

    vecs = _pack_cols(w["b1"], w["g1"], w["be1"], bvo,
                      w["b2"], w["g2"], w["be2"],
                      w["b3"], w["g3"], w["be3"],
                      w["b4"], w["g4"], w["be4"],
                      w["b5"], w["g5"], w["be5"],
                      w["bd1"], w["gd1"], w["bed1"],
                      w["bd2"], w["gd2"], w["bed2"],
                      w["bd3"], w["bm"], z0,
                      np.full(128, LN_EPS, np.float32))

    def b16(x):
        return np.ascontiguousarray(x.astype(ml_dtypes.bfloat16))

    wd["id"] = np.eye(128, dtype=np.float32)
    wd["vecs"] = vecs
    blob_off, nblob = kb.blob_layout()
    wblob = np.zeros((128, nblob), ml_dtypes.bfloat16)
    for name, (off, ntiles, M) in blob_off.items():
        if name == "xt":
            continue
        wsrc = np.asarray(wd[name], np.float32)
        for k in range(ntiles):
            wblob[:, off + k * M:off + (k + 1) * M] = b16(
                wsrc[k * 128:(k + 1) * 128, :])

    seqs = np.asarray(inputs["sequences"], np.float32)
    xt_off, xnt, xm = blob_off["xt"]
    in_maps = []
    for c in range(NCORES):
        xs = seqs[c * BL:(c + 1) * BL, :S, :]
        xt = b16(xs.reshape(BL * S, D).T)           # [D, NTOK]
        blob = wblob.copy()
        for k in range(xnt):
            blob[:, xt_off + k * xm:xt_off + (k + 1) * xm] = \
                xt[k * 128:(k + 1) * 128, :]
        in_maps.append(dict(blob16=blob))
    return in_maps


def _l2_term(inputs):
    names = ["W1", "b1", "g1", "be1", "Wv", "bv", "Wo", "bo", "W2", "b2", "g2",
             "be2", "W3", "b3", "g3", "be3", "W4", "b4", "g4", "be4", "W5",
             "b5", "g5", "be5", "Wd1", "bd1", "gd1", "bed1", "Wd2", "bd2",
             "gd2", "bed2", "Wd3", "bd3", "Wm", "Um", "bm"]
    l2 = sum(np.linalg.norm(np.asarray(inputs[n], np.float64)) for n in names)
    return float(np.clip(l2, 0.0, 10.0))


def _combine(kb, res, inputs):
    tok = np.concatenate([res.results[c]["tok_loss"].reshape(-1)
                          for c in range(NCORES)])
    l2 = _l2_term(inputs)
    per_tok = np.clip(tok.astype(np.float64) + 1e-4 * l2, 0.0, 100.0)
    nb = kb.BL * NCORES
    return np.float32(per_tok.sum() / nb)


def kernel(**inputs):
    seqs = np.asarray(inputs["sequences"])
    S = seqs.shape[1]
    BL = seqs.shape[0] // NCORES
    kb = _get_built(S, BL)
    in_maps = _host_inputs(kb, inputs)
    res = run_bass_kernel_spmd(kb.nc, in_maps, list(range(NCORES)))
    return _combine(kb, res, inputs)
